# revision 14
# baseline (speedup 1.0000x reference)
"""DepletionLSTM Trainium2 kernel (v2).

Self-contained: builds a Bass/Tile kernel for the 2-layer-LSTM network,
shards the batch over 8 NeuronCores (pure data parallelism), runs via
PJRT/axon, returns the full [8192, 30] float32 output.

Strategy (per core, 1024 batch):
- All activations SBUF-resident; zero in-loop DRAM traffic.
- The input-projection LayerNorm is folded INTO the layer-0 gate weights:
  x0 = (W_in x + b_in - mu 1) r  ==  [W'|b'] @ [x r ; r]  with
  W' = W_in - 1 ws^T/H, b' = b_in - bs/H, so the layer-0 input-gate matmul
  uses an 8-row stationary Stat0 = ([W'|b'])^T diag(g_in) Wih0^T and the
  8-row moving operand xt = [x r ; r].  No separate projection matmul, no
  x0 tile, no PSUM->SBUF projection copy.
- rstd (r) is applied in batch-major layout BEFORE the PE transpose: a Pool
  (gpsimd) op scales x_t[128p, 8q, 7f] by rT[:, :, t] (0-stride broadcast
  over f) and writes r itself into lane 7, then 8 PE transposes produce the
  [8, BL] fp16 moving operand.  All per-step DRAM broadcast DMAs are gone.
- fp16 everywhere on matmul operands and the elementwise chain: DVE runs in
  2x mode (594ns per [128,1024] op vs 1127 fp32); cell state c stays fp32.
- LN stats prepass in [T, BL] layout via the quadratic-form identity; the
  per-partition stat constants are broadcast with one-time rank-1 PE
  matmuls (DRAM staging only for the [1,49] row reload).
- Layer 1 runs TWO timesteps behind layer 0: every ACT op in a steady-state
  period then depends only on >=half-period-old results, so the h0
  recurrence tail (tanh -> h-mult -> PE -> first gate ACT) hides entirely
  under L1's gate ops -- the ACT engine runs gap-free at its 10x1038ns/step
  floor.  All gate activations live in one ACT table (no table loads).

PSUM: "pg" gates 3x[128,1024] (6 banks), "px" x-transposes 2x[8,512]
(2 banks); prepass uses a separate pool that closes before the loop.
"""
import sys
sys.path.insert(0, '/opt/trn_rl_repo')

import numpy as np

B, T, F, H, D1, D2, OUT = 8192, 90, 7, 128, 128, 64, 30
NCORES = 8
BL = B // NCORES
G4 = 4 * H
NH = BL // 512
QB = BL // 128
EPS = 1e-5
C_F16 = True


def _build(nc, T_steps=T, dbg=False):
    import concourse.tile as tile
    from concourse import mybir
    from concourse.masks import make_identity

    f32 = mybir.dt.float32
    f16 = mybir.dt.float16
    AF = mybir.ActivationFunctionType
    ALU = mybir.AluOpType
    cdt = f16 if C_F16 else f32

    # ---------------- DRAM I/O ----------------
    x_d = nc.dram_tensor("x", [BL, T, F], f32, kind="ExternalInput")
    W_in_d = nc.dram_tensor("W_in", [H, F], f32, kind="ExternalInput")
    b_in_d = nc.dram_tensor("b_in", [H], f32, kind="ExternalInput")
    g_in_d = nc.dram_tensor("g_in", [H], f32, kind="ExternalInput")
    be_in_d = nc.dram_tensor("be_in", [H], f32, kind="ExternalInput")
    Wih_d = [nc.dram_tensor("Wih0", [G4, H], f32, kind="ExternalInput"),
             nc.dram_tensor("Wih1", [G4, H], f32, kind="ExternalInput")]
    Whh_d = [nc.dram_tensor("Whh0", [G4, H], f32, kind="ExternalInput"),
             nc.dram_tensor("Whh1", [G4, H], f32, kind="ExternalInput")]
    bih_d = [nc.dram_tensor("bih0", [G4], f32, kind="ExternalInput"),
             nc.dram_tensor("bih1", [G4], f32, kind="ExternalInput")]
    bhh_d = [nc.dram_tensor("bhh0", [G4], f32, kind="ExternalInput"),
             nc.dram_tensor("bhh1", [G4], f32, kind="ExternalInput")]
    g_ln_d = nc.dram_tensor("g_ln", [H], f32, kind="ExternalInput")
    be_ln_d = nc.dram_tensor("be_ln", [H], f32, kind="ExternalInput")
    W_d1_d = nc.dram_tensor("W_d1", [D1, H], f32, kind="ExternalInput")
    b_d1_d = nc.dram_tensor("b_d1", [D1], f32, kind="ExternalInput")
    W_d2_d = nc.dram_tensor("W_d2", [D2, D1], f32, kind="ExternalInput")
    b_d2_d = nc.dram_tensor("b_d2", [D2], f32, kind="ExternalInput")
    W_d3_d = nc.dram_tensor("W_d3", [OUT, D2], f32, kind="ExternalInput")
    b_d3_d = nc.dram_tensor("b_d3", [OUT], f32, kind="ExternalInput")
    out_d = nc.dram_tensor("out", [BL, OUT], f32, kind="ExternalOutput")
    if dbg:
        dbg_xt = nc.dram_tensor("dbg_xt", [8, BL], f32, kind="ExternalOutput")
        dbg_h0 = nc.dram_tensor("dbg_h0", [H, BL], f32, kind="ExternalOutput")
        dbg_c0 = nc.dram_tensor("dbg_c0", [H, BL], f32, kind="ExternalOutput")
        dbg_r = nc.dram_tensor("dbg_r", [T, BL], f32, kind="ExternalOutput")

    import contextlib
    with tile.TileContext(nc) as tc, contextlib.ExitStack() as ctx:
        singles = ctx.enter_context(tc.tile_pool(name="singles", bufs=1))
        trans = ctx.enter_context(tc.tile_pool(name="trans", bufs=2))
        small = ctx.enter_context(tc.tile_pool(name="small", bufs=2))
        xsp = ctx.enter_context(tc.tile_pool(name="xsp", bufs=3))
        dpool = ctx.enter_context(tc.tile_pool(name="dpool", bufs=1, space="DRAM"))

        # ---------------- constants ----------------
        ident = singles.tile([128, 128], f32)
        make_identity(nc, ident)
        ident16 = singles.tile([128, 128], f16)
        make_identity(nc, ident16)
        ones_row = singles.tile([1, 512], f32)
        nc.vector.memset(ones_row, 1.0)
        ones_col = singles.tile([128, 1], f32)
        nc.vector.memset(ones_col, 1.0)
        ones_col16 = singles.tile([128, 1], f16)
        nc.vector.memset(ones_col16, 1.0)
        ones_row90 = singles.tile([1, T], f32)
        nc.vector.memset(ones_row90, 1.0)
        ones_row128_16 = singles.tile([1, 128], f16)
        nc.vector.memset(ones_row128_16, 1.0)
        eps_col = singles.tile([T, 1], f32)
        nc.vector.memset(eps_col, EPS)
        ones_q = singles.tile([128, QB], f32)
        nc.vector.memset(ones_q, 1.0)

        def load_col(dram_vec, n, name):
            t_ = singles.tile([n, 1], f32, name=name, tag=name)
            nc.sync.dma_start(out=t_, in_=dram_vec[:].rearrange("(p o) -> p o", o=1))
            return t_

        g_in_c = load_col(g_in_d, H, "g_in_c")
        be_in_c = load_col(be_in_d, H, "be_in_c")
        b_in_c = load_col(b_in_d, H, "b_in_c")
        g_ln_c = load_col(g_ln_d, H, "g_ln_c")
        be_ln_c = load_col(be_ln_d, H, "be_ln_c")
        b_d1_c = load_col(b_d1_d, D1, "b_d1_c")
        b_d2_c = load_col(b_d2_d, D2, "b_d2_c")
        b_d3_c = load_col(b_d3_d, OUT, "b_d3_c")

        # ---------------- x loads ----------------
        # xq[p, q, t, f] = x[128q+p, t, f]  (contiguous 2520B runs per (p,q))
        xq = singles.tile([128, QB, T, F], f32)
        nc.sync.dma_start(
            out=xq, in_=x_d[:, :, :].rearrange("(q p) t f -> p q t f", p=128))
        xqh = singles.tile([128, QB, T, F], f16)
        # prepass layout: x_tm[t, q, p, f] = x[128q+p, t, f] (built by PE
        # transposes from xq below -- a direct DMA needs 92k descriptors)
        x_tm = singles.tile([T, QB, 128, F], f32)

        # ------- weights: load + PE-transpose; LN fold into layer-0 -------
        with tc.tile_pool(name="ps_pre", bufs=3, space="PSUM") as pre:
            def transpose_to(dst, src_ap, p, fdim):
                pt = pre.tile([fdim, p], f32, tag="scr", name="tr_ps")
                nc.tensor.transpose(pt, src_ap, ident[:p, :p])
                nc.vector.tensor_copy(out=dst, in_=pt)

            w_in_raw = singles.tile([H, F], f32)
            nc.sync.dma_start(out=w_in_raw, in_=W_in_d[:, :])

            # stat-constant matmuls + DRAM staging FIRST so sbc lands early
            p_m = pre.tile([F, F], f32, tag="scr", name="stat_m")
            nc.tensor.matmul(p_m, w_in_raw, w_in_raw, start=True, stop=True)
            p_l = pre.tile([1, F + 2], f32, tag="scr", name="stat_l")
            nc.tensor.matmul(p_l[:, 0:F], b_in_c, w_in_raw, start=True,
                             stop=False, skip_group_check=True)
            nc.tensor.matmul(p_l[:, F:F + 1], b_in_c, b_in_c, start=False,
                             stop=False, skip_group_check=True)
            nc.tensor.matmul(p_l[:, F + 1:F + 2], ones_col, b_in_c, start=False,
                             stop=True, skip_group_check=True)
            p_ws = pre.tile([1, F + 1], f32, tag="scr", name="p_ws")
            nc.tensor.matmul(p_ws[:, 0:F], ones_col, w_in_raw, start=True,
                             stop=False, skip_group_check=True)
            nc.tensor.matmul(p_ws[:, F:F + 1], ones_col, b_in_c, start=False,
                             stop=True, skip_group_check=True)
            wsn = small.tile([1, F + 1], f32, tag="wsn", name="wsn")
            nc.vector.tensor_scalar_mul(out=wsn, in0=p_ws, scalar1=1.0 / H)
            m_sb = small.tile([F, F], f32, tag="m_sb", name="m_sb")
            nc.vector.tensor_copy(out=m_sb, in_=p_m)
            l_sb = small.tile([1, F + 2], f32, tag="l_sb", name="l_sb")
            nc.vector.tensor_copy(out=l_sb, in_=p_l)
            NST = F * F + (F + 2) + (F + 1)
            stat_dram = dpool.tile([F + 2, F * F], f32)
            nc.sync.dma_start(
                out=stat_dram[0:1, :].rearrange("o (a b) -> (o a) b", a=F),
                in_=m_sb)
            nc.sync.dma_start(out=stat_dram[F:F + 1, 0:F + 2], in_=l_sb)
            nc.sync.dma_start(out=stat_dram[F + 1:F + 2, 0:F + 1], in_=wsn)
            srow = singles.tile([1, NST], f32)
            nc.sync.dma_start(out=srow[:, 0:F * F],
                              in_=stat_dram[0:1, :])
            nc.sync.dma_start(out=srow[:, F * F:F * F + F + 2],
                              in_=stat_dram[F:F + 1, 0:F + 2])
            nc.sync.dma_start(out=srow[:, F * F + F + 2:NST],
                              in_=stat_dram[F + 1:F + 2, 0:F + 1])
            sbc_ps = pre.tile([T, NST], f32, tag="sbc", name="sbc_ps", bufs=1)
            nc.tensor.matmul(sbc_ps, ones_row90, srow, start=True, stop=True)
            sbc = singles.tile([T, NST], f32)
            nc.vector.tensor_copy(out=sbc, in_=sbc_ps)

            # x_tm[t, q, :, fi] = transpose(xq[:, q, :, fi]) -- 56 PE
            # transposes; PSUM->SBUF copies alternate DVE/Pool
            for fi in range(F):
                for q in range(QB):
                    prx = pre.tile([T, 128], f32, tag="scr", name="prx")
                    nc.tensor.transpose(prx, xq[:, q, :, fi], ident)
                    if (fi * QB + q) % 2 == 0:
                        nc.vector.tensor_copy(out=x_tm[:, q, :, fi], in_=prx)
                    else:
                        nc.scalar.activation(out=x_tm[:, q, :, fi], in_=prx,
                                             func=AF.Copy, scale=1.0)

            wihT0f = singles.tile([H, 4, H], f32)  # raw Wih0^T per gate
            wihT1 = singles.tile([H, 4, H], f16)
            whhT = [singles.tile([H, 4, H], f16, name=f"whhT{L}", tag=f"whhT{L}")
                    for L in range(2)]
            for L in range(2):
                for cc in range(4):
                    raw = trans.tile([H, H], f32, tag="u", name="raw")
                    nc.sync.dma_start(out=raw, in_=Wih_d[L][cc * H:(cc + 1) * H, :])
                    pt_w = pre.tile([H, H], f32, tag="scr", name="tr_ps_w")
                    nc.tensor.transpose(pt_w, raw, ident)
                    if L == 0:
                        nc.vector.tensor_copy(out=wihT0f[:, cc, :], in_=pt_w)
                    else:
                        nc.vector.tensor_copy(out=wihT1[:, cc, :], in_=pt_w)
                    raw2 = trans.tile([H, H], f32, tag="v_", name="raw2")
                    nc.sync.dma_start(out=raw2, in_=Whh_d[L][cc * H:(cc + 1) * H, :])
                    transpose_to(whhT[L][:, cc, :], raw2, H, H)

            # gate biases beff[L] [128, 4]; layer-0 gains Wih0 @ be_in
            beff = []
            for L in range(2):
                bt_ = singles.tile([H, 4], f32, name=f"beff{L}", tag=f"beff{L}")
                bih_sb = small.tile([H, 4], f32, tag="bload", name="bih_sb")
                nc.sync.dma_start(out=bih_sb,
                                  in_=bih_d[L][:].rearrange("(c p) -> p c", p=H))
                bhh_sb = small.tile([H, 4], f32, tag="bload2", name="bhh_sb")
                nc.sync.dma_start(out=bhh_sb,
                                  in_=bhh_d[L][:].rearrange("(c p) -> p c", p=H))
                nc.vector.tensor_add(out=bt_, in0=bih_sb, in1=bhh_sb)
                beff.append(bt_)
            for cc in range(4):
                pb = pre.tile([H, 1], f32, tag="scr", name="pb")
                nc.tensor.matmul(pb, wihT0f[:, cc, :], be_in_c, start=True,
                                 stop=True)
                nc.vector.tensor_add(out=beff[0][:, cc:cc + 1],
                                     in0=beff[0][:, cc:cc + 1], in1=pb)

            # ---- LN fold: Pg = diag(g_in) [W_in - 1 ws^T/H | b_in - bs/H] ----
            pw_bc = pre.tile([H, F + 1], f32, tag="scr", name="pw_bc")
            nc.tensor.matmul(pw_bc, ones_row[:, 0:H], wsn, start=True, stop=True)
            cat8 = small.tile([H, F + 1], f32, tag="cat8", name="cat8")
            nc.vector.tensor_copy(out=cat8[:, 0:F], in_=w_in_raw)
            nc.vector.tensor_copy(out=cat8[:, F:F + 1], in_=b_in_c)
            Pg = singles.tile([H, F + 1], f32)
            nc.vector.tensor_sub(out=Pg, in0=cat8, in1=pw_bc)
            nc.vector.tensor_scalar_mul(out=Pg, in0=Pg, scalar1=g_in_c)
            stat0 = singles.tile([F + 1, 4, H], f16)
            for cc in range(4):
                ps8 = pre.tile([F + 1, H], f32, tag="scr", name="ps8")
                nc.tensor.matmul(ps8, Pg, wihT0f[:, cc, :], start=True, stop=True)
                nc.vector.tensor_copy(out=stat0[:, cc, :], in_=ps8)

            # dense head weights (transposed, f16 stationaries)
            wd1T = singles.tile([H, D1], f16)
            wd1_raw = trans.tile([D1, H], f32, tag="u", name="wd1_raw")
            nc.sync.dma_start(out=wd1_raw, in_=W_d1_d[:, :])
            transpose_to(wd1T, wd1_raw, D1, H)
            wd2T = singles.tile([D1, D2], f16)
            wd2_raw = trans.tile([D2, D1], f32, tag="v_", name="wd2_raw")
            nc.sync.dma_start(out=wd2_raw, in_=W_d2_d[:, :])
            transpose_to(wd2T, wd2_raw, D2, D1)
            wd3T = singles.tile([D2, OUT], f16)
            wd3_raw = trans.tile([OUT, D2], f32, tag="u", name="wd3_raw")
            nc.sync.dma_start(out=wd3_raw, in_=W_d3_d[:, :])
            transpose_to(wd3T, wd3_raw, OUT, D2)

            # ------------- prepass: LN stats in [T, BL] layout -------------
            # p' = W_in x + b_in per (h | b,t); over h:
            #   sum p'   = ws . x + bs
            #   sum p'^2 = x^T M x + 2 l^T x + c0,  M = W^T W, l = W^T b
            mbc = sbc[:, 0:F * F]
            lbc = sbc[:, F * F:F * F + F]
            c0bc = sbc[:, F * F + F:F * F + F + 1]
            bsbc = sbc[:, F * F + F + 1:F * F + F + 2]  # sum b (NOT /H)
            wbc = sbc[:, F * F + F + 2:F * F + F + 2 + F]  # ws/H
            bshbc = sbc[:, NST - 1:NST]  # bs/H

            def xf(fi):
                return x_tm[:T_steps, :, :, fi].rearrange("t q p -> t (q p)")

            TS = T_steps
            nmu_all = singles.tile([T, BL], f32)
            r_all = singles.tile([T, BL], f32)
            acc = trans.tile([T, BL], f32, tag="sig_i", name="st_acc")
            nc.vector.tensor_scalar_mul(out=acc[:TS], in0=xf(0),
                                        scalar1=wbc[:TS, 0:1])
            for fi in range(1, F):
                nc.vector.scalar_tensor_tensor(
                    out=acc[:TS], in0=xf(fi), scalar=wbc[:TS, fi:fi + 1],
                    in1=acc[:TS], op0=ALU.mult, op1=ALU.add)
            # acc now = ws.x/H ; nmu = -(acc + bs/H)
            nc.vector.tensor_scalar(out=nmu_all[:TS], in0=acc[:TS],
                                    scalar1=bshbc[:TS], scalar2=-1.0,
                                    op0=ALU.add, op1=ALU.mult)
            # quadratic form: y-chains (TensorScalarPtr, DVE-only) on DVE;
            # the x*y products and qacc accumulation (plain TensorTensor) on
            # Pool. Emission inline keeps the conservative cross-engine sem
            # joins tight.
            qacc = trans.tile([T, BL], f32, tag="x_pool_a", name="st_qacc")
            tprod = trans.tile([T, BL], f32, tag="x_pool_b", name="st_tprod")
            for fi in range(F):
                yf = trans.tile([T, BL], f32, tag="st_yf", name="st_yf")
                nc.vector.tensor_scalar_mul(out=yf[:TS], in0=xf(0),
                                            scalar1=mbc[:TS, fi * F:fi * F + 1])
                for fj in range(1, F):
                    nc.vector.scalar_tensor_tensor(
                        out=yf[:TS], in0=xf(fj),
                        scalar=mbc[:TS, fi * F + fj:fi * F + fj + 1],
                        in1=yf[:TS], op0=ALU.mult, op1=ALU.add)
                if fi == 0:
                    nc.gpsimd.tensor_tensor(out=qacc[:TS], in0=xf(fi),
                                            in1=yf[:TS], op=ALU.mult)
                else:
                    nc.gpsimd.tensor_tensor(out=tprod[:TS], in0=xf(fi),
                                            in1=yf[:TS], op=ALU.mult)
                    nc.gpsimd.tensor_add(out=qacc[:TS], in0=qacc[:TS],
                                         in1=tprod[:TS])
            # + 2 l.x
            lin = trans.tile([T, BL], f32, tag="st_lin", name="st_lin")
            nc.vector.tensor_scalar_mul(out=lin[:TS], in0=xf(0),
                                        scalar1=lbc[:TS, 0:1])
            for fi in range(1, F):
                nc.vector.scalar_tensor_tensor(
                    out=lin[:TS], in0=xf(fi), scalar=lbc[:TS, fi:fi + 1],
                    in1=lin[:TS], op0=ALU.mult, op1=ALU.add)
            nc.vector.scalar_tensor_tensor(out=qacc[:TS], in0=lin[:TS],
                                           scalar=2.0, in1=qacc[:TS],
                                           op0=ALU.mult, op1=ALU.add)
            # var = (q + c0)/H - mu^2 ; r = 1/sqrt(var+eps)
            nc.vector.tensor_scalar(out=qacc[:TS], in0=qacc[:TS],
                                    scalar1=c0bc[:TS], scalar2=1.0 / H,
                                    op0=ALU.add, op1=ALU.mult)
            musq = trans.tile([T, BL], f32, tag="st_yf", name="st_musq")
            nc.vector.tensor_tensor(out=musq[:TS], in0=nmu_all[:TS],
                                    in1=nmu_all[:TS], op=ALU.mult)
            nc.vector.tensor_sub(out=qacc[:TS], in0=qacc[:TS], in1=musq[:TS])
            nc.scalar.activation(out=r_all[:TS], in_=qacc[:TS], func=AF.Sqrt,
                                 bias=eps_col[:TS], scale=1.0)
            nc.vector.reciprocal(out=r_all[:TS], in_=r_all[:TS])
            if dbg:
                nc.sync.dma_start(out=dbg_r[:TS, :], in_=r_all[:TS])
            # f32 -> f16 x copy for the loop (Pool; runs during the DVE tail)
            nc.gpsimd.tensor_tensor(
                out=xqh[:, :, :, :].rearrange("p q t f -> p (q t f)"),
                in0=xq[:, :, :, :].rearrange("p q t f -> p (q t f)"),
                in1=ones_col[:, 0:1].to_broadcast([128, QB * T * F]),
                op=ALU.mult)
            # rT[p, q, t] = r[t, 128q+p]  (batch-major rstd for the scale op)
            rT = singles.tile([128, QB, T], f32)
            for q in range(QB):
                prt = pre.tile([128, T], f32, tag="scr", name="prt")
                nc.tensor.transpose(prt[:, :TS], r_all[:TS, q * 128:(q + 1) * 128],
                                    ident[:TS, :TS])
                nc.vector.tensor_copy(out=rT[:, q, :TS], in_=prt[:, :TS])

        # ---------------- states ----------------
        h1 = singles.tile([H, BL], f16, name="h1", tag="h1")
        c = [singles.tile([H, BL], cdt, name="c0", tag="c0"),
             singles.tile([H, BL], cdt, name="c1", tag="c1")]
        h0_z = trans.tile([H, BL], f16, tag="h0", name="h0_init", bufs=3)
        nc.vector.memset(h0_z, 0.0)
        nc.vector.memset(h1, 0.0)
        for L in range(2):
            nc.vector.memset(c[L], 0.0)
        # layer-1 runs TWO steps behind layer-0: every ACT op in a period then
        # depends only on >= half-period-old results, so the h0 recurrence
        # tail (tanh -> h-mult -> PE -> first gate ACT) hides under L1's ops.
        h0_hist = [None, h0_z]

        ps_pg = ctx.enter_context(tc.tile_pool(name="ps_pg", bufs=3, space="PSUM"))
        ps_px = ctx.enter_context(tc.tile_pool(name="ps_px", bufs=2, space="PSUM"))

        def pg_tile(shape, name):
            return ps_pg.tile(shape, f32, tag="pg", name=name)

        # ---------------- main loop ----------------
        def lstm_step(L, inp, hprev, hout, hh_first, split=False,
                      first_gate_split=False):
            sig_i = trans.tile([H, BL], f16, tag="sig_i", name="sig_i")
            sig_f = trans.tile([H, BL], f16, tag="sig_f", name="sig_f")
            tg = trans.tile([H, BL], f16, tag="tg", name="tg")
            sig_o = trans.tile([H, BL], f16, tag="sig_o", name="sig_o")
            outs = [sig_i, sig_f, tg, sig_o]
            funcs = [AF.Sigmoid, AF.Sigmoid, AF.Tanh, AF.Sigmoid]
            wih = stat0 if L == 0 else wihT1
            for gc in range(4):
                pg = pg_tile([H, BL], "pg_gates")
                for hc in range(NH):
                    sl = slice(hc * 512, (hc + 1) * 512)
                    ops = [(wih[:, gc, :], inp), (whhT[L][:, gc, :], hprev)]
                    if hh_first:
                        ops.reverse()
                    nc.tensor.matmul(pg[:, sl], ops[0][0], ops[0][1][:, sl],
                                     start=True, stop=False)
                    nc.tensor.matmul(pg[:, sl], ops[1][0], ops[1][1][:, sl],
                                     start=False, stop=True)
                if gc == 0 and first_gate_split:
                    for hc in range(NH):
                        sl = slice(hc * 512, (hc + 1) * 512)
                        nc.scalar.activation(out=outs[gc][:, sl],
                                             in_=pg[:, sl], func=funcs[gc],
                                             bias=beff[L][:, gc:gc + 1],
                                             scale=1.0)
                else:
                    nc.scalar.activation(out=outs[gc], in_=pg, func=funcs[gc],
                                         bias=beff[L][:, gc:gc + 1], scale=1.0)
            u = trans.tile([H, BL], f16, tag="u", name="u")
            v_ = trans.tile([H, BL], cdt, tag="v_", name="v_")
            tc_ = trans.tile([H, BL], f16, tag="tc_", name="tc_")
            if not split:
                nc.vector.tensor_tensor(out=u, in0=sig_i, in1=tg, op=ALU.mult)
                nc.vector.tensor_tensor(out=v_, in0=sig_f, in1=c[L], op=ALU.mult)
                nc.vector.tensor_add(out=c[L], in0=u, in1=v_)
                nc.scalar.activation(out=tc_, in_=c[L], func=AF.Tanh, scale=1.0)
                nc.vector.tensor_tensor(out=hout, in0=sig_o, in1=tc_, op=ALU.mult)
            else:
                # half-column tail: lets tanh/h pipeline against the DVE chain
                # v-products first: they only need sig_f (2nd ACT op)
                for hc in range(NH):
                    sl = slice(hc * 512, (hc + 1) * 512)
                    nc.vector.tensor_tensor(out=v_[:, sl], in0=sig_f[:, sl],
                                            in1=c[L][:, sl], op=ALU.mult)
                for hc in range(NH):
                    sl = slice(hc * 512, (hc + 1) * 512)
                    nc.vector.tensor_tensor(out=u[:, sl], in0=sig_i[:, sl],
                                            in1=tg[:, sl], op=ALU.mult)
                    nc.vector.tensor_add(out=c[L][:, sl], in0=u[:, sl],
                                         in1=v_[:, sl])
                for hc in range(NH):
                    sl = slice(hc * 512, (hc + 1) * 512)
                    nc.scalar.activation(out=tc_[:, sl], in_=c[L][:, sl],
                                         func=AF.Tanh, scale=1.0)
                for hc in range(NH):
                    sl = slice(hc * 512, (hc + 1) * 512)
                    nc.vector.tensor_tensor(out=hout[:, sl], in0=sig_o[:, sl],
                                            in1=tc_[:, sl], op=ALU.mult)

        for t in range(T_steps):
            # scale x_t by rstd in batch-major layout (Pool), lane 7 = rstd
            xs = xsp.tile([128, QB, F + 1], f16, tag="xs", name="xs")
            nc.gpsimd.tensor_tensor(
                out=xs[:, :, 0:F], in0=xqh[:, :, t, :],
                in1=rT[:, :, t:t + 1].to_broadcast([128, QB, F]), op=ALU.mult)
            nc.gpsimd.tensor_tensor(out=xs[:, :, F], in0=rT[:, :, t],
                                    in1=ones_q, op=ALU.mult)
            # PE transpose to [8, BL] fp16 moving operand xt = [x r ; r]
            xt = trans.tile([F + 1, BL], f16, tag="xt", name="xt")
            for half in range(2):
                px = ps_px.tile([F + 1, 512], f16, tag="pxt", name="pxt")
                for qi in range(4):
                    q = half * 4 + qi
                    nc.tensor.transpose(px[:, qi * 128:(qi + 1) * 128],
                                        xs[:, q, :], ident16)
                nc.vector.tensor_copy(
                    out=xt[:, half * 512:(half + 1) * 512], in_=px)
            if t > 1:
                lstm_step(1, h0_hist[0], h1, h1, hh_first=True)
            h0_new = trans.tile([H, BL], f16, tag="h0", name="h0_new", bufs=3)
            lstm_step(0, xt, h0_hist[1], h0_new, hh_first=False, split=True)
            h0_hist = [h0_hist[1], h0_new]
            if dbg and t == 0:
                xtc = trans.tile([F + 1, BL], f32, tag="v_", name="xtc_dbg")
                nc.vector.tensor_copy(out=xtc, in_=xt)
                nc.sync.dma_start(out=dbg_xt[:, :], in_=xtc)
                h0c = trans.tile([H, BL], f32, tag="u", name="h0c_dbg")
                nc.vector.tensor_copy(out=h0c, in_=h0_new)
                nc.sync.dma_start(out=dbg_h0[:, :], in_=h0c)
                c0c = trans.tile([H, BL], f32, tag="tc_", name="c0c_dbg")
                nc.vector.tensor_copy(out=c0c, in_=c[0])
                nc.sync.dma_start(out=dbg_c0[:, :], in_=c0c)
        lstm_step(1, h0_hist[0], h1, h1, hh_first=True)
        lstm_step(1, h0_hist[1], h1, h1, hh_first=True)

        # ---------------- head ----------------
        sqh = trans.tile([H, BL], f16, tag="sig_f", name="sqh")
        nc.vector.tensor_tensor(out=sqh, in0=h1, in1=h1, op=ALU.mult)
        ps_s1 = pg_tile([1, BL], "ps_s1")
        ps_s2 = pg_tile([1, BL], "ps_s2")
        for hc in range(NH):
            sl = slice(hc * 512, (hc + 1) * 512)
            nc.tensor.matmul(ps_s1[:, sl], ones_col16, h1[:, sl],
                             start=True, stop=True, skip_group_check=True)
            nc.tensor.matmul(ps_s2[:, sl], ones_col16, sqh[:, sl],
                             start=True, stop=True, skip_group_check=True)
        nmu_h = singles.tile([1, BL], f32, tag="nmu_h", name="nmu_h")
        nc.vector.tensor_scalar_mul(out=nmu_h, in0=ps_s1, scalar1=-1.0 / H)
        musq_h = singles.tile([1, BL], f32, tag="musq", name="musq_h")
        nc.vector.tensor_tensor(out=musq_h, in0=nmu_h, in1=nmu_h, op=ALU.mult)
        v_h = singles.tile([1, BL], f32, tag="v_h", name="v_h")
        nc.vector.tensor_scalar_mul(out=v_h, in0=ps_s2, scalar1=1.0 / H)
        nc.vector.tensor_sub(out=v_h, in0=v_h, in1=musq_h)
        nc.scalar.activation(out=v_h, in_=v_h, func=AF.Sqrt,
                             bias=eps_col[0:1], scale=1.0)
        nc.vector.reciprocal(out=v_h, in_=v_h)
        nm16 = singles.tile([1, BL], f16, tag="nm16", name="nm16")
        nc.vector.tensor_copy(out=nm16, in_=nmu_h)
        rh16 = singles.tile([1, BL], f16, tag="rh16", name="rh16")
        nc.vector.tensor_copy(out=rh16, in_=v_h)
        pnm = pg_tile([H, BL], "pnm")
        prh = ps_px.tile([H, 512], f32, tag="pxt", name="prh0")
        prh2 = ps_px.tile([H, 512], f32, tag="pxt", name="prh1")
        prhs = [prh, prh2]
        for hc in range(NH):
            sl = slice(hc * 512, (hc + 1) * 512)
            nc.tensor.matmul(pnm[:, sl], ones_row128_16, nm16[:, sl],
                             start=True, stop=True, skip_group_check=True)
            nc.tensor.matmul(prhs[hc], ones_row128_16, rh16[:, sl],
                             start=True, stop=True, skip_group_check=True)
        t1 = trans.tile([H, BL], f32, tag="tg", name="t1")
        nc.vector.tensor_tensor(out=t1, in0=h1, in1=pnm, op=ALU.add)
        t2 = trans.tile([H, BL], f32, tag="sig_o", name="t2")
        for hc in range(NH):
            sl = slice(hc * 512, (hc + 1) * 512)
            nc.vector.tensor_tensor(out=t2[:, sl], in0=t1[:, sl], in1=prhs[hc],
                                    op=ALU.mult)
        last = trans.tile([H, BL], f16, tag="u", name="last")
        nc.vector.tensor_scalar(out=last, in0=t2, scalar1=g_ln_c,
                                scalar2=be_ln_c, op0=ALU.mult, op1=ALU.add)
        pd1 = pg_tile([D1, BL], "pd1")
        for hc in range(NH):
            sl = slice(hc * 512, (hc + 1) * 512)
            nc.tensor.matmul(pd1[:, sl], wd1T, last[:, sl], start=True, stop=True,
                             skip_group_check=True)
        d1 = trans.tile([D1, BL], f16, tag="v_", name="d1")
        nc.scalar.activation(out=d1, in_=pd1, func=AF.Relu, bias=b_d1_c, scale=1.0)
        pd2 = pg_tile([D2, BL], "pd2")
        for hc in range(NH):
            sl = slice(hc * 512, (hc + 1) * 512)
            nc.tensor.matmul(pd2[:, sl], wd2T, d1[:, sl], start=True, stop=True,
                             skip_group_check=True)
        d2 = trans.tile([D2, BL], f16, tag="tc_", name="d2")
        nc.scalar.activation(out=d2, in_=pd2, func=AF.Relu, bias=b_d2_c, scale=1.0)
        pd3 = pg_tile([OUT, BL], "pd3")
        for hc in range(NH):
            sl = slice(hc * 512, (hc + 1) * 512)
            nc.tensor.matmul(pd3[:, sl], wd3T, d2[:, sl], start=True, stop=True,
                             skip_group_check=True)
        o3 = trans.tile([OUT, BL], f32, tag="sig_f", name="o3")
        nc.scalar.activation(out=o3, in_=pd3, func=AF.Identity, bias=b_d3_c,
                             scale=1.0)
        outT = singles.tile([128, QB, OUT], f32)
        for q in range(QB):
            pot = ps_px.tile([128, OUT], f32, tag="pxt", name="pot")
            nc.tensor.transpose(pot, o3[:, q * 128:(q + 1) * 128],
                                ident[:OUT, :OUT])
            nc.vector.tensor_copy(out=outT[:, q, :], in_=pot)
        nc.sync.dma_start(
            out=out_d[:, :].rearrange("(q p) c -> p q c", p=128),
            in_=outT)
    return nc


_CACHE = {}


def _get_runner(T_steps=T):
    if "runner" in _CACHE:
        return _CACHE["runner"]
    import jax
    from jax.sharding import Mesh, PartitionSpec
    from jax.experimental.shard_map import shard_map
    import concourse.bacc as bacc
    import concourse.mybir as mybir
    from concourse.bass2jax import install_neuronx_cc_hook, _bass_exec_p, \
        partition_id_tensor

    nc = bacc.Bacc()
    _build(nc, T_steps=T_steps)
    nc.compile()
    install_neuronx_cc_hook()

    partition_name = nc.partition_id_tensor.name if nc.partition_id_tensor else None
    in_names, out_names, out_avals, zero_outs = [], [], [], []
    for alloc in nc.m.functions[0].allocations:
        if not isinstance(alloc, mybir.MemoryLocationSet):
            continue
        name = alloc.memorylocations[0].name
        if alloc.kind == "ExternalInput":
            if name != partition_name:
                in_names.append(name)
        elif alloc.kind == "ExternalOutput":
            out_names.append(name)
            shape = tuple(alloc.tensor_shape)
            dtype = mybir.dt.np(alloc.dtype)
            out_avals.append(jax.core.ShapedArray(shape, dtype))
            zero_outs.append(np.zeros(shape, dtype))
    n_params = len(in_names)
    all_in_names = in_names + out_names + ([partition_name] if partition_name else [])

    def _body(*args):
        operands = list(args)
        if partition_name is not None:
            operands.append(partition_id_tensor())
        outs = _bass_exec_p.bind(
            *operands,
            out_avals=tuple(out_avals),
            in_names=tuple(all_in_names),
            out_names=tuple(out_names),
            lowering_input_output_aliases=(),
            sim_require_finite=False,
            sim_require_nnan=False,
            nc=nc,
        )
        return tuple(outs)

    devices = jax.devices()[:NCORES]
    mesh = Mesh(np.asarray(devices), ("core",))
    in_specs = (PartitionSpec("core"),) * (n_params + len(out_names))
    out_specs = (PartitionSpec("core"),) * len(out_names)
    sharded = jax.jit(
        shard_map(_body, mesh=mesh, in_specs=in_specs, out_specs=out_specs,
                  check_rep=False),
        keep_unused=True)
    _CACHE["runner"] = (sharded, in_names, out_names, zero_outs)
    return _CACHE["runner"]


def kernel(**inputs) -> np.ndarray:
    sharded, in_names, out_names, zero_outs = _get_runner()
    inp = {k: np.ascontiguousarray(np.asarray(v), dtype=np.float32)
           for k, v in inputs.items()}

    def core_val(name, ci):
        if name == "x":
            return inp["x"][ci * BL:(ci + 1) * BL]
        return inp[name]

    concat_in = [
        np.concatenate([core_val(n, ci) for ci in range(NCORES)], axis=0)
        for n in in_names
    ]
    concat_zeros = [
        np.zeros((NCORES * z.shape[0], *z.shape[1:]), z.dtype) for z in zero_outs
    ]
    import jax
    out_arrs = sharded(*concat_in, *concat_zeros)
    jax.block_until_ready(out_arrs)
    oi = out_names.index("out")
    full = np.asarray(out_arrs[oi]).reshape(B, OUT)
    return full.astype(np.float32)


# revision 19
# speedup vs baseline: 1.0507x; 1.0507x over previous
"""DepletionLSTM Trainium2 kernel (v2).

Self-contained: builds a Bass/Tile kernel for the 2-layer-LSTM network,
shards the batch over 8 NeuronCores (pure data parallelism), runs via
PJRT/axon, returns the full [8192, 30] float32 output.

Strategy (per core, 1024 batch):
- All activations SBUF-resident; zero in-loop DRAM traffic.
- The input-projection LayerNorm is folded INTO the layer-0 gate weights:
  x0 = (W_in x + b_in - mu 1) r  ==  [W'|b'] @ [x r ; r]  with
  W' = W_in - 1 ws^T/H, b' = b_in - bs/H, so the layer-0 input-gate matmul
  uses an 8-row stationary Stat0 = ([W'|b'])^T diag(g_in) Wih0^T and the
  8-row moving operand xt = [x r ; r].  No separate projection matmul, no
  x0 tile, no PSUM->SBUF projection copy.
- rstd (r) is applied in batch-major layout BEFORE the PE transpose: a Pool
  (gpsimd) op scales x_t[128p, 8q, 7f] by rT[:, :, t] (0-stride broadcast
  over f) and writes r itself into lane 7, then 8 PE transposes produce the
  [8, BL] fp16 moving operand.  All per-step DRAM broadcast DMAs are gone.
- fp16 everywhere on matmul operands and the elementwise chain: DVE runs in
  2x mode (594ns per [128,1024] op vs 1127 fp32); cell state c stays fp32.
- LN stats prepass in [T, BL] layout via the quadratic-form identity; the
  per-partition stat constants are broadcast with one-time rank-1 PE
  matmuls (DRAM staging only for the [1,49] row reload).
- Layer 1 runs TWO timesteps behind layer 0: every ACT op in a steady-state
  period then depends only on >=half-period-old results, so the h0
  recurrence tail (tanh -> h-mult -> PE -> first gate ACT) hides entirely
  under L1's gate ops -- the ACT engine runs gap-free at its 10x1038ns/step
  floor.  All gate activations live in one ACT table (no table loads).

PSUM: "pg" gates 3x[128,1024] (6 banks), "px" x-transposes 2x[8,512]
(2 banks); prepass uses a separate pool that closes before the loop.
"""
import sys
sys.path.insert(0, '/opt/trn_rl_repo')

import numpy as np

B, T, F, H, D1, D2, OUT = 8192, 90, 7, 128, 128, 64, 30
NCORES = 8
BL = B // NCORES
G4 = 4 * H
NH = BL // 512
QB = BL // 128
EPS = 1e-5
C_F16 = True


def _build(nc, T_steps=T, dbg=False):
    import concourse.tile as tile
    from concourse import mybir
    from concourse.masks import make_identity

    f32 = mybir.dt.float32
    f16 = mybir.dt.float16
    AF = mybir.ActivationFunctionType
    ALU = mybir.AluOpType
    cdt = f16 if C_F16 else f32

    # ---------------- DRAM I/O ----------------
    x_d = nc.dram_tensor("x", [BL, T, F], f32, kind="ExternalInput")
    W_in_d = nc.dram_tensor("W_in", [H, F], f32, kind="ExternalInput")
    b_in_d = nc.dram_tensor("b_in", [H], f32, kind="ExternalInput")
    g_in_d = nc.dram_tensor("g_in", [H], f32, kind="ExternalInput")
    be_in_d = nc.dram_tensor("be_in", [H], f32, kind="ExternalInput")
    Wih_d = [nc.dram_tensor("Wih0", [G4, H], f32, kind="ExternalInput"),
             nc.dram_tensor("Wih1", [G4, H], f32, kind="ExternalInput")]
    Whh_d = [nc.dram_tensor("Whh0", [G4, H], f32, kind="ExternalInput"),
             nc.dram_tensor("Whh1", [G4, H], f32, kind="ExternalInput")]
    bih_d = [nc.dram_tensor("bih0", [G4], f32, kind="ExternalInput"),
             nc.dram_tensor("bih1", [G4], f32, kind="ExternalInput")]
    bhh_d = [nc.dram_tensor("bhh0", [G4], f32, kind="ExternalInput"),
             nc.dram_tensor("bhh1", [G4], f32, kind="ExternalInput")]
    g_ln_d = nc.dram_tensor("g_ln", [H], f32, kind="ExternalInput")
    be_ln_d = nc.dram_tensor("be_ln", [H], f32, kind="ExternalInput")
    W_d1_d = nc.dram_tensor("W_d1", [D1, H], f32, kind="ExternalInput")
    b_d1_d = nc.dram_tensor("b_d1", [D1], f32, kind="ExternalInput")
    W_d2_d = nc.dram_tensor("W_d2", [D2, D1], f32, kind="ExternalInput")
    b_d2_d = nc.dram_tensor("b_d2", [D2], f32, kind="ExternalInput")
    W_d3_d = nc.dram_tensor("W_d3", [OUT, D2], f32, kind="ExternalInput")
    b_d3_d = nc.dram_tensor("b_d3", [OUT], f32, kind="ExternalInput")
    out_d = nc.dram_tensor("out", [BL, OUT], f32, kind="ExternalOutput")
    if dbg:
        dbg_xt = nc.dram_tensor("dbg_xt", [8, BL], f32, kind="ExternalOutput")
        dbg_h0 = nc.dram_tensor("dbg_h0", [H, BL], f32, kind="ExternalOutput")
        dbg_c0 = nc.dram_tensor("dbg_c0", [H, BL], f32, kind="ExternalOutput")
        dbg_r = nc.dram_tensor("dbg_r", [T, BL], f32, kind="ExternalOutput")

    import contextlib
    with tile.TileContext(nc) as tc, contextlib.ExitStack() as ctx:
        singles = ctx.enter_context(tc.tile_pool(name="singles", bufs=1))
        trans = ctx.enter_context(tc.tile_pool(name="trans", bufs=2))
        small = ctx.enter_context(tc.tile_pool(name="small", bufs=2))
        xsp = ctx.enter_context(tc.tile_pool(name="xsp", bufs=3))
        dpool = ctx.enter_context(tc.tile_pool(name="dpool", bufs=1, space="DRAM"))

        # ---------------- constants ----------------
        ident = singles.tile([128, 128], f32)
        make_identity(nc, ident)
        ident16 = singles.tile([128, 128], f16)
        make_identity(nc, ident16)
        ones_row = singles.tile([1, 512], f32)
        nc.vector.memset(ones_row, 1.0)
        ones_col = singles.tile([128, 1], f32)
        nc.vector.memset(ones_col, 1.0)
        ones_col16 = singles.tile([128, 1], f16)
        nc.vector.memset(ones_col16, 1.0)
        ones_row90 = singles.tile([1, T], f32)
        nc.vector.memset(ones_row90, 1.0)
        ones_row128_16 = singles.tile([1, 128], f16)
        nc.vector.memset(ones_row128_16, 1.0)
        eps_col = singles.tile([128, 1], f32)
        nc.vector.memset(eps_col, EPS)
        ones_q = singles.tile([128, QB], f32)
        nc.vector.memset(ones_q, 1.0)

        def load_col(dram_vec, n, name):
            t_ = singles.tile([n, 1], f32, name=name, tag=name)
            nc.sync.dma_start(out=t_, in_=dram_vec[:].rearrange("(p o) -> p o", o=1))
            return t_

        w_in_raw = singles.tile([H, F], f32)
        nc.sync.dma_start(out=w_in_raw, in_=W_in_d[:, :])
        b_in_c = load_col(b_in_d, H, "b_in_c")
        g_in_c = load_col(g_in_d, H, "g_in_c")
        be_in_c = load_col(be_in_d, H, "be_in_c")
        g_ln_c = load_col(g_ln_d, H, "g_ln_c")
        be_ln_c = load_col(be_ln_d, H, "be_ln_c")
        b_d1_c = load_col(b_d1_d, D1, "b_d1_c")
        b_d2_c = load_col(b_d2_d, D2, "b_d2_c")
        b_d3_c = load_col(b_d3_d, OUT, "b_d3_c")

        # ---------------- x loads ----------------
        # xq[p, q, t, f] = x[128q+p, t, f]  (contiguous 2520B runs per (p,q))
        xq = singles.tile([128, QB, T, F], f32)
        nc.sync.dma_start(
            out=xq, in_=x_d[:, :, :].rearrange("(q p) t f -> p q t f", p=128))
        xqh = singles.tile([128, QB, T, F], f16)

        # ------- weights: load + PE-transpose; LN fold into layer-0 -------
        with tc.tile_pool(name="ps_pre", bufs=3, space="PSUM") as pre:
            def transpose_to(dst, src_ap, p, fdim):
                pt = pre.tile([fdim, p], f32, tag="scr", name="tr_ps")
                nc.tensor.transpose(pt, src_ap, ident[:p, :p])
                nc.vector.tensor_copy(out=dst, in_=pt)

            # stat constants, all-partition broadcast WITHOUT a DRAM
            # roundtrip: rhs columns hold per-h products; contracting with an
            # all-ones [128,128] stationary sums over h into every partition.
            NST = F * F + (F + 2) + (F + 1)
            rhs_all = small.tile([H, NST], f32, tag="rhs_all", name="rhs_all")
            for i in range(F):
                nc.vector.tensor_tensor(
                    out=rhs_all[:, i * F:(i + 1) * F], in0=w_in_raw,
                    in1=w_in_raw[:, i:i + 1].to_broadcast([H, F]), op=ALU.mult)
            nc.vector.tensor_scalar_mul(out=rhs_all[:, F * F:F * F + F],
                                        in0=w_in_raw, scalar1=b_in_c)
            nc.vector.tensor_tensor(out=rhs_all[:, F * F + F:F * F + F + 1],
                                    in0=b_in_c, in1=b_in_c, op=ALU.mult)
            nc.vector.tensor_scalar_mul(
                out=rhs_all[:, F * F + F + 1:F * F + F + 2], in0=b_in_c,
                scalar1=1.0)
            nc.vector.tensor_scalar_mul(
                out=rhs_all[:, F * F + F + 2:F * F + F + 2 + F], in0=w_in_raw,
                scalar1=1.0 / H)
            nc.vector.tensor_scalar_mul(out=rhs_all[:, NST - 1:NST],
                                        in0=b_in_c, scalar1=1.0 / H)
            ones128 = singles.tile([128, 128], f32)
            nc.vector.memset(ones128, 1.0)
            sbc_ps = pre.tile([128, NST], f32, tag="sbc", name="sbc_ps",
                              bufs=1)
            nc.tensor.matmul(sbc_ps, ones128, rhs_all, start=True, stop=True)
            # p_ws/wsn (partition-0 row) still needed for the LN weight fold
            p_ws = pre.tile([1, F + 1], f32, tag="scr", name="p_ws")
            nc.tensor.matmul(p_ws[:, 0:F], ones_col, w_in_raw, start=True,
                             stop=False, skip_group_check=True)
            nc.tensor.matmul(p_ws[:, F:F + 1], ones_col, b_in_c, start=False,
                             stop=True, skip_group_check=True)
            wsn = small.tile([1, F + 1], f32, tag="wsn", name="wsn")
            nc.vector.tensor_scalar_mul(out=wsn, in0=p_ws, scalar1=1.0 / H)
            sbc = singles.tile([128, NST], f32)
            nc.vector.tensor_copy(out=sbc, in_=sbc_ps)

            wihT0f = singles.tile([H, 4, H], f32)  # raw Wih0^T per gate
            wihT1 = singles.tile([H, 4, H], f16)
            whhT = [singles.tile([H, 4, H], f16, name=f"whhT{L}", tag=f"whhT{L}")
                    for L in range(2)]
            for L in range(2):
                for cc in range(4):
                    raw = trans.tile([H, H], f32, tag="u", name="raw")
                    nc.sync.dma_start(out=raw, in_=Wih_d[L][cc * H:(cc + 1) * H, :])
                    pt_w = pre.tile([H, H], f32, tag="scr", name="tr_ps_w")
                    nc.tensor.transpose(pt_w, raw, ident)
                    if L == 0:
                        nc.vector.tensor_copy(out=wihT0f[:, cc, :], in_=pt_w)
                    else:
                        nc.vector.tensor_copy(out=wihT1[:, cc, :], in_=pt_w)
                    raw2 = trans.tile([H, H], f32, tag="v_", name="raw2")
                    nc.sync.dma_start(out=raw2, in_=Whh_d[L][cc * H:(cc + 1) * H, :])
                    transpose_to(whhT[L][:, cc, :], raw2, H, H)

            # gate biases beff[L] [128, 4]; layer-0 gains Wih0 @ be_in
            beff = []
            for L in range(2):
                bt_ = singles.tile([H, 4], f32, name=f"beff{L}", tag=f"beff{L}")
                bih_sb = small.tile([H, 4], f32, tag="bload", name="bih_sb")
                nc.sync.dma_start(out=bih_sb,
                                  in_=bih_d[L][:].rearrange("(c p) -> p c", p=H))
                bhh_sb = small.tile([H, 4], f32, tag="bload2", name="bhh_sb")
                nc.sync.dma_start(out=bhh_sb,
                                  in_=bhh_d[L][:].rearrange("(c p) -> p c", p=H))
                nc.vector.tensor_add(out=bt_, in0=bih_sb, in1=bhh_sb)
                beff.append(bt_)
            for cc in range(4):
                pb = pre.tile([H, 1], f32, tag="scr", name="pb")
                nc.tensor.matmul(pb, wihT0f[:, cc, :], be_in_c, start=True,
                                 stop=True)
                nc.vector.tensor_add(out=beff[0][:, cc:cc + 1],
                                     in0=beff[0][:, cc:cc + 1], in1=pb)

            # ---- LN fold: Pg = diag(g_in) [W_in - 1 ws^T/H | b_in - bs/H] ----
            pw_bc = pre.tile([H, F + 1], f32, tag="scr", name="pw_bc")
            nc.tensor.matmul(pw_bc, ones_row[:, 0:H], wsn, start=True, stop=True)
            cat8 = small.tile([H, F + 1], f32, tag="cat8", name="cat8")
            nc.vector.tensor_copy(out=cat8[:, 0:F], in_=w_in_raw)
            nc.vector.tensor_copy(out=cat8[:, F:F + 1], in_=b_in_c)
            Pg = singles.tile([H, F + 1], f32)
            nc.vector.tensor_sub(out=Pg, in0=cat8, in1=pw_bc)
            nc.vector.tensor_scalar_mul(out=Pg, in0=Pg, scalar1=g_in_c)
            stat0 = singles.tile([F + 1, 4, H], f16)
            for cc in range(4):
                ps8 = pre.tile([F + 1, H], f32, tag="scr", name="ps8")
                nc.tensor.matmul(ps8, Pg, wihT0f[:, cc, :], start=True, stop=True)
                nc.vector.tensor_copy(out=stat0[:, cc, :], in_=ps8)

            # dense head weights (transposed, f16 stationaries)
            wd1T = singles.tile([H, D1], f16)
            wd1_raw = trans.tile([D1, H], f32, tag="u", name="wd1_raw")
            nc.sync.dma_start(out=wd1_raw, in_=W_d1_d[:, :])
            transpose_to(wd1T, wd1_raw, D1, H)
            wd2T = singles.tile([D1, D2], f16)
            wd2_raw = trans.tile([D2, D1], f32, tag="v_", name="wd2_raw")
            nc.sync.dma_start(out=wd2_raw, in_=W_d2_d[:, :])
            transpose_to(wd2T, wd2_raw, D2, D1)
            wd3T = singles.tile([D2, OUT], f16)
            wd3_raw = trans.tile([OUT, D2], f32, tag="u", name="wd3_raw")
            nc.sync.dma_start(out=wd3_raw, in_=W_d3_d[:, :])
            transpose_to(wd3T, wd3_raw, OUT, D2)

            # -------- prepass: LN stats in batch-major [128, (q t)] --------
            # per (t,b) sample:  sum_h p = ws.x + bs ;
            #   sum_h p^2 = x^T M x + 2 l^T x + c0  (M = W^T W, l = W^T b).
            # The stat scalars are per-sample CONSTANTS, so the stats run
            # directly on xq's own [128p, (q t)] layout -- no transposes --
            # and rstd lands directly in rT's batch-major layout.
            mbc = sbc[:, 0:F * F]
            lbc = sbc[:, F * F:F * F + F]
            c0bc = sbc[:, F * F + F:F * F + F + 1]
            wbc = sbc[:, F * F + F + 2:F * F + F + 2 + F]  # ws/H
            bshbc = sbc[:, NST - 1:NST]  # bs/H

            TS = T_steps

            def xf(fi):
                return xq[:, :, :TS, fi]

            rT = singles.tile([128, QB, T], f32)
            nmu = trans.tile([128, QB, T], f32, tag="sig_i", name="st_nmu")
            nc.vector.tensor_scalar_mul(out=nmu[:, :, :TS], in0=xf(0),
                                        scalar1=wbc[:, 0:1])
            for fi in range(1, F):
                nc.vector.scalar_tensor_tensor(
                    out=nmu[:, :, :TS], in0=xf(fi), scalar=wbc[:, fi:fi + 1],
                    in1=nmu[:, :, :TS], op0=ALU.mult, op1=ALU.add)
            # nmu = -(ws.x/H + bs/H)
            nc.vector.tensor_scalar(out=nmu[:, :, :TS], in0=nmu[:, :, :TS],
                                    scalar1=bshbc, scalar2=-1.0,
                                    op0=ALU.add, op1=ALU.mult)
            # quadratic form: y-chains (TensorScalarPtr, DVE-only) on DVE;
            # the x*y products and qacc accumulation (plain TensorTensor) on
            # Pool. Emission inline keeps the conservative cross-engine sem
            # joins tight.
            qacc = trans.tile([128, QB, T], f32, tag="x_pool_a", name="st_qacc")
            tprod = trans.tile([128, QB, T], f32, tag="x_pool_b",
                               name="st_tprod")
            for fi in range(F):
                yf = trans.tile([128, QB, T], f32, tag="st_yf", name="st_yf")
                nc.vector.tensor_scalar_mul(out=yf[:, :, :TS], in0=xf(0),
                                            scalar1=mbc[:, fi * F:fi * F + 1])
                for fj in range(1, F):
                    nc.vector.scalar_tensor_tensor(
                        out=yf[:, :, :TS], in0=xf(fj),
                        scalar=mbc[:, fi * F + fj:fi * F + fj + 1],
                        in1=yf[:, :, :TS], op0=ALU.mult, op1=ALU.add)
                if fi == 0:
                    nc.gpsimd.tensor_tensor(out=qacc[:, :, :TS], in0=xf(fi),
                                            in1=yf[:, :, :TS], op=ALU.mult)
                else:
                    nc.gpsimd.tensor_tensor(out=tprod[:, :, :TS], in0=xf(fi),
                                            in1=yf[:, :, :TS], op=ALU.mult)
                    nc.gpsimd.tensor_add(out=qacc[:, :, :TS],
                                         in0=qacc[:, :, :TS],
                                         in1=tprod[:, :, :TS])
            # + 2 l.x
            lin = trans.tile([128, QB, T], f32, tag="st_lin", name="st_lin")
            nc.vector.tensor_scalar_mul(out=lin[:, :, :TS], in0=xf(0),
                                        scalar1=lbc[:, 0:1])
            for fi in range(1, F):
                nc.vector.scalar_tensor_tensor(
                    out=lin[:, :, :TS], in0=xf(fi), scalar=lbc[:, fi:fi + 1],
                    in1=lin[:, :, :TS], op0=ALU.mult, op1=ALU.add)
            nc.vector.scalar_tensor_tensor(out=qacc[:, :, :TS],
                                           in0=lin[:, :, :TS],
                                           scalar=2.0, in1=qacc[:, :, :TS],
                                           op0=ALU.mult, op1=ALU.add)
            # var = (q + c0)/H - mu^2 ; rT = 1/sqrt(var+eps)
            nc.vector.tensor_scalar(out=qacc[:, :, :TS], in0=qacc[:, :, :TS],
                                    scalar1=c0bc, scalar2=1.0 / H,
                                    op0=ALU.add, op1=ALU.mult)
            musq = trans.tile([128, QB, T], f32, tag="st_yf", name="st_musq")
            nc.vector.tensor_tensor(out=musq[:, :, :TS], in0=nmu[:, :, :TS],
                                    in1=nmu[:, :, :TS], op=ALU.mult)
            nc.vector.tensor_sub(out=qacc[:, :, :TS], in0=qacc[:, :, :TS],
                                 in1=musq[:, :, :TS])
            nc.scalar.activation(out=rT[:, :, :TS], in_=qacc[:, :, :TS],
                                 func=AF.Sqrt, bias=eps_col, scale=1.0)
            nc.vector.reciprocal(out=rT[:, :, :TS], in_=rT[:, :, :TS])
            # f32 -> f16 x copy for the loop (Pool; runs during the DVE tail)
            nc.gpsimd.tensor_tensor(
                out=xqh[:, :, :, :].rearrange("p q t f -> p (q t f)"),
                in0=xq[:, :, :, :].rearrange("p q t f -> p (q t f)"),
                in1=ones_col[:, 0:1].to_broadcast([128, QB * T * F]),
                op=ALU.mult)

        # ---------------- states ----------------
        h1 = singles.tile([H, BL], f16, name="h1", tag="h1")
        c = [singles.tile([H, BL], cdt, name="c0", tag="c0"),
             singles.tile([H, BL], cdt, name="c1", tag="c1")]
        h0_z = trans.tile([H, BL], f16, tag="h0", name="h0_init", bufs=3)
        nc.vector.memset(h0_z, 0.0)
        nc.vector.memset(h1, 0.0)
        for L in range(2):
            nc.vector.memset(c[L], 0.0)
        # layer-1 runs TWO steps behind layer-0: every ACT op in a period then
        # depends only on >= half-period-old results, so the h0 recurrence
        # tail (tanh -> h-mult -> PE -> first gate ACT) hides under L1's ops.
        h0_hist = [None, h0_z]

        ps_pg = ctx.enter_context(tc.tile_pool(name="ps_pg", bufs=3, space="PSUM"))
        ps_px = ctx.enter_context(tc.tile_pool(name="ps_px", bufs=2, space="PSUM"))

        def pg_tile(shape, name):
            return ps_pg.tile(shape, f32, tag="pg", name=name)

        # ---------------- main loop ----------------
        def lstm_step(L, inp, hprev, hout, hh_first, split=False,
                      first_gate_split=False):
            sig_i = trans.tile([H, BL], f16, tag="sig_i", name="sig_i")
            sig_f = trans.tile([H, BL], f16, tag="sig_f", name="sig_f")
            tg = trans.tile([H, BL], f16, tag="tg", name="tg")
            sig_o = trans.tile([H, BL], f16, tag="sig_o", name="sig_o")
            outs = [sig_i, sig_f, tg, sig_o]
            funcs = [AF.Sigmoid, AF.Sigmoid, AF.Tanh, AF.Sigmoid]
            wih = stat0 if L == 0 else wihT1
            for gc in range(4):
                pg = pg_tile([H, BL], "pg_gates")
                for hc in range(NH):
                    sl = slice(hc * 512, (hc + 1) * 512)
                    ops = [(wih[:, gc, :], inp), (whhT[L][:, gc, :], hprev)]
                    if hh_first:
                        ops.reverse()
                    nc.tensor.matmul(pg[:, sl], ops[0][0], ops[0][1][:, sl],
                                     start=True, stop=False)
                    nc.tensor.matmul(pg[:, sl], ops[1][0], ops[1][1][:, sl],
                                     start=False, stop=True)
                if gc == 0 and first_gate_split:
                    for hc in range(NH):
                        sl = slice(hc * 512, (hc + 1) * 512)
                        nc.scalar.activation(out=outs[gc][:, sl],
                                             in_=pg[:, sl], func=funcs[gc],
                                             bias=beff[L][:, gc:gc + 1],
                                             scale=1.0)
                else:
                    nc.scalar.activation(out=outs[gc], in_=pg, func=funcs[gc],
                                         bias=beff[L][:, gc:gc + 1], scale=1.0)
            u = trans.tile([H, BL], f16, tag="u", name="u")
            v_ = trans.tile([H, BL], cdt, tag="v_", name="v_")
            tc_ = trans.tile([H, BL], f16, tag="tc_", name="tc_")
            if not split:
                nc.vector.tensor_tensor(out=v_, in0=sig_f, in1=c[L], op=ALU.mult)
                nc.vector.tensor_tensor(out=u, in0=sig_i, in1=tg, op=ALU.mult)
                nc.vector.tensor_add(out=c[L], in0=u, in1=v_)
                nc.scalar.activation(out=tc_, in_=c[L], func=AF.Tanh, scale=1.0)
                nc.vector.tensor_tensor(out=hout, in0=sig_o, in1=tc_, op=ALU.mult)
            else:
                # half-column tail: lets tanh/h pipeline against the DVE chain
                # v-products first: they only need sig_f (2nd ACT op)
                for hc in range(NH):
                    sl = slice(hc * 512, (hc + 1) * 512)
                    nc.vector.tensor_tensor(out=v_[:, sl], in0=sig_f[:, sl],
                                            in1=c[L][:, sl], op=ALU.mult)
                for hc in range(NH):
                    sl = slice(hc * 512, (hc + 1) * 512)
                    nc.vector.tensor_tensor(out=u[:, sl], in0=sig_i[:, sl],
                                            in1=tg[:, sl], op=ALU.mult)
                    nc.vector.tensor_add(out=c[L][:, sl], in0=u[:, sl],
                                         in1=v_[:, sl])
                for hc in range(NH):
                    sl = slice(hc * 512, (hc + 1) * 512)
                    nc.scalar.activation(out=tc_[:, sl], in_=c[L][:, sl],
                                         func=AF.Tanh, scale=1.0)
                for hc in range(NH):
                    sl = slice(hc * 512, (hc + 1) * 512)
                    nc.vector.tensor_tensor(out=hout[:, sl], in0=sig_o[:, sl],
                                            in1=tc_[:, sl], op=ALU.mult)

        for t in range(T_steps):
            # scale x_t by rstd in batch-major layout (Pool), lane 7 = rstd
            xs = xsp.tile([128, QB, F + 1], f16, tag="xs", name="xs")
            nc.gpsimd.tensor_tensor(
                out=xs[:, :, 0:F], in0=xqh[:, :, t, :],
                in1=rT[:, :, t:t + 1].to_broadcast([128, QB, F]), op=ALU.mult)
            nc.gpsimd.tensor_tensor(out=xs[:, :, F], in0=rT[:, :, t],
                                    in1=ones_q, op=ALU.mult)
            # PE transpose to [8, BL] fp16 moving operand xt = [x r ; r]
            xt = trans.tile([F + 1, BL], f16, tag="xt", name="xt")
            for half in range(2):
                px = ps_px.tile([F + 1, 512], f16, tag="pxt", name="pxt")
                for qi in range(4):
                    q = half * 4 + qi
                    nc.tensor.transpose(px[:, qi * 128:(qi + 1) * 128],
                                        xs[:, q, :], ident16)
                nc.vector.tensor_copy(
                    out=xt[:, half * 512:(half + 1) * 512], in_=px)
            if t > 1:
                lstm_step(1, h0_hist[0], h1, h1, hh_first=True)
            h0_new = trans.tile([H, BL], f16, tag="h0", name="h0_new", bufs=3)
            lstm_step(0, xt, h0_hist[1], h0_new, hh_first=False, split=False)
            h0_hist = [h0_hist[1], h0_new]
            if dbg and t == 0:
                xtc = trans.tile([F + 1, BL], f32, tag="v_", name="xtc_dbg")
                nc.vector.tensor_copy(out=xtc, in_=xt)
                nc.sync.dma_start(out=dbg_xt[:, :], in_=xtc)
                h0c = trans.tile([H, BL], f32, tag="u", name="h0c_dbg")
                nc.vector.tensor_copy(out=h0c, in_=h0_new)
                nc.sync.dma_start(out=dbg_h0[:, :], in_=h0c)
                c0c = trans.tile([H, BL], f32, tag="tc_", name="c0c_dbg")
                nc.vector.tensor_copy(out=c0c, in_=c[0])
                nc.sync.dma_start(out=dbg_c0[:, :], in_=c0c)
        lstm_step(1, h0_hist[0], h1, h1, hh_first=True, split=True,
                  first_gate_split=True)
        lstm_step(1, h0_hist[1], h1, h1, hh_first=True, split=True,
                  first_gate_split=True)

        # ---------------- head ----------------
        sqh = trans.tile([H, BL], f16, tag="sig_f", name="sqh")
        nc.vector.tensor_tensor(out=sqh, in0=h1, in1=h1, op=ALU.mult)
        ps_s1 = pg_tile([1, BL], "ps_s1")
        ps_s2 = pg_tile([1, BL], "ps_s2")
        for hc in range(NH):
            sl = slice(hc * 512, (hc + 1) * 512)
            nc.tensor.matmul(ps_s1[:, sl], ones_col16, h1[:, sl],
                             start=True, stop=True, skip_group_check=True)
            nc.tensor.matmul(ps_s2[:, sl], ones_col16, sqh[:, sl],
                             start=True, stop=True, skip_group_check=True)
        nmu_h = singles.tile([1, BL], f32, tag="nmu_h", name="nmu_h")
        nc.vector.tensor_scalar_mul(out=nmu_h, in0=ps_s1, scalar1=-1.0 / H)
        musq_h = singles.tile([1, BL], f32, tag="musq", name="musq_h")
        nc.vector.tensor_tensor(out=musq_h, in0=nmu_h, in1=nmu_h, op=ALU.mult)
        v_h = singles.tile([1, BL], f32, tag="v_h", name="v_h")
        nc.vector.tensor_scalar_mul(out=v_h, in0=ps_s2, scalar1=1.0 / H)
        nc.vector.tensor_sub(out=v_h, in0=v_h, in1=musq_h)
        nc.scalar.activation(out=v_h, in_=v_h, func=AF.Sqrt,
                             bias=eps_col[0:1], scale=1.0)
        nc.vector.reciprocal(out=v_h, in_=v_h)
        nm16 = singles.tile([1, BL], f16, tag="nm16", name="nm16")
        nc.vector.tensor_copy(out=nm16, in_=nmu_h)
        rh16 = singles.tile([1, BL], f16, tag="rh16", name="rh16")
        nc.vector.tensor_copy(out=rh16, in_=v_h)
        pnm = pg_tile([H, BL], "pnm")
        prh = ps_px.tile([H, 512], f32, tag="pxt", name="prh0")
        prh2 = ps_px.tile([H, 512], f32, tag="pxt", name="prh1")
        prhs = [prh, prh2]
        for hc in range(NH):
            sl = slice(hc * 512, (hc + 1) * 512)
            nc.tensor.matmul(pnm[:, sl], ones_row128_16, nm16[:, sl],
                             start=True, stop=True, skip_group_check=True)
            nc.tensor.matmul(prhs[hc], ones_row128_16, rh16[:, sl],
                             start=True, stop=True, skip_group_check=True)
        t1 = trans.tile([H, BL], f32, tag="tg", name="t1")
        nc.vector.tensor_tensor(out=t1, in0=h1, in1=pnm, op=ALU.add)
        t2 = trans.tile([H, BL], f32, tag="sig_o", name="t2")
        for hc in range(NH):
            sl = slice(hc * 512, (hc + 1) * 512)
            nc.vector.tensor_tensor(out=t2[:, sl], in0=t1[:, sl], in1=prhs[hc],
                                    op=ALU.mult)
        last = trans.tile([H, BL], f16, tag="u", name="last")
        nc.vector.tensor_scalar(out=last, in0=t2, scalar1=g_ln_c,
                                scalar2=be_ln_c, op0=ALU.mult, op1=ALU.add)
        pd1 = pg_tile([D1, BL], "pd1")
        for hc in range(NH):
            sl = slice(hc * 512, (hc + 1) * 512)
            nc.tensor.matmul(pd1[:, sl], wd1T, last[:, sl], start=True, stop=True,
                             skip_group_check=True)
        d1 = trans.tile([D1, BL], f16, tag="v_", name="d1")
        nc.scalar.activation(out=d1, in_=pd1, func=AF.Relu, bias=b_d1_c, scale=1.0)
        pd2 = pg_tile([D2, BL], "pd2")
        for hc in range(NH):
            sl = slice(hc * 512, (hc + 1) * 512)
            nc.tensor.matmul(pd2[:, sl], wd2T, d1[:, sl], start=True, stop=True,
                             skip_group_check=True)
        d2 = trans.tile([D2, BL], f16, tag="tc_", name="d2")
        nc.scalar.activation(out=d2, in_=pd2, func=AF.Relu, bias=b_d2_c, scale=1.0)
        pd3 = pg_tile([OUT, BL], "pd3")
        for hc in range(NH):
            sl = slice(hc * 512, (hc + 1) * 512)
            nc.tensor.matmul(pd3[:, sl], wd3T, d2[:, sl], start=True, stop=True,
                             skip_group_check=True)
        o3 = trans.tile([OUT, BL], f32, tag="sig_f", name="o3")
        nc.scalar.activation(out=o3, in_=pd3, func=AF.Identity, bias=b_d3_c,
                             scale=1.0)
        outT = singles.tile([128, QB, OUT], f32)
        for q in range(QB):
            pot = ps_px.tile([128, OUT], f32, tag="pxt", name="pot")
            nc.tensor.transpose(pot, o3[:, q * 128:(q + 1) * 128],
                                ident[:OUT, :OUT])
            nc.vector.tensor_copy(out=outT[:, q, :], in_=pot)
        nc.sync.dma_start(
            out=out_d[:, :].rearrange("(q p) c -> p q c", p=128),
            in_=outT)
    return nc


_CACHE = {}


def _get_runner(T_steps=T):
    if "runner" in _CACHE:
        return _CACHE["runner"]
    import jax
    from jax.sharding import Mesh, PartitionSpec
    from jax.experimental.shard_map import shard_map
    import concourse.bacc as bacc
    import concourse.mybir as mybir
    from concourse.bass2jax import install_neuronx_cc_hook, _bass_exec_p, \
        partition_id_tensor

    nc = bacc.Bacc()
    _build(nc, T_steps=T_steps)
    nc.compile()
    install_neuronx_cc_hook()

    partition_name = nc.partition_id_tensor.name if nc.partition_id_tensor else None
    in_names, out_names, out_avals, zero_outs = [], [], [], []
    for alloc in nc.m.functions[0].allocations:
        if not isinstance(alloc, mybir.MemoryLocationSet):
            continue
        name = alloc.memorylocations[0].name
        if alloc.kind == "ExternalInput":
            if name != partition_name:
                in_names.append(name)
        elif alloc.kind == "ExternalOutput":
            out_names.append(name)
            shape = tuple(alloc.tensor_shape)
            dtype = mybir.dt.np(alloc.dtype)
            out_avals.append(jax.core.ShapedArray(shape, dtype))
            zero_outs.append(np.zeros(shape, dtype))
    n_params = len(in_names)
    all_in_names = in_names + out_names + ([partition_name] if partition_name else [])

    def _body(*args):
        operands = list(args)
        if partition_name is not None:
            operands.append(partition_id_tensor())
        outs = _bass_exec_p.bind(
            *operands,
            out_avals=tuple(out_avals),
            in_names=tuple(all_in_names),
            out_names=tuple(out_names),
            lowering_input_output_aliases=(),
            sim_require_finite=False,
            sim_require_nnan=False,
            nc=nc,
        )
        return tuple(outs)

    devices = jax.devices()[:NCORES]
    mesh = Mesh(np.asarray(devices), ("core",))
    in_specs = (PartitionSpec("core"),) * (n_params + len(out_names))
    out_specs = (PartitionSpec("core"),) * len(out_names)
    sharded = jax.jit(
        shard_map(_body, mesh=mesh, in_specs=in_specs, out_specs=out_specs,
                  check_rep=False),
        keep_unused=True)
    _CACHE["runner"] = (sharded, in_names, out_names, zero_outs)
    return _CACHE["runner"]


def kernel(**inputs) -> np.ndarray:
    sharded, in_names, out_names, zero_outs = _get_runner()
    inp = {k: np.ascontiguousarray(np.asarray(v), dtype=np.float32)
           for k, v in inputs.items()}

    def core_val(name, ci):
        if name == "x":
            return inp["x"][ci * BL:(ci + 1) * BL]
        return inp[name]

    concat_in = [
        np.concatenate([core_val(n, ci) for ci in range(NCORES)], axis=0)
        for n in in_names
    ]
    concat_zeros = [
        np.zeros((NCORES * z.shape[0], *z.shape[1:]), z.dtype) for z in zero_outs
    ]
    import jax
    out_arrs = sharded(*concat_in, *concat_zeros)
    jax.block_until_ready(out_arrs)
    oi = out_names.index("out")
    full = np.asarray(out_arrs[oi]).reshape(B, OUT)
    return full.astype(np.float32)


# revision 29
# speedup vs baseline: 1.0538x; 1.0030x over previous
"""DepletionLSTM Trainium2 kernel (v2).

Self-contained: builds a Bass/Tile kernel for the 2-layer-LSTM network,
shards the batch over 8 NeuronCores (pure data parallelism), runs via
PJRT/axon, returns the full [8192, 30] float32 output.

Strategy (per core, 1024 batch):
- All activations SBUF-resident; zero in-loop DRAM traffic.
- The input-projection LayerNorm is folded INTO the layer-0 gate weights:
  x0 = (W_in x + b_in - mu 1) r  ==  [W'|b'] @ [x r ; r]  with
  W' = W_in - 1 ws^T/H, b' = b_in - bs/H, so the layer-0 input-gate matmul
  uses an 8-row stationary Stat0 = ([W'|b'])^T diag(g_in) Wih0^T and the
  8-row moving operand xt = [x r ; r].  No separate projection matmul, no
  x0 tile, no PSUM->SBUF projection copy.
- rstd (r) is applied in batch-major layout BEFORE the PE transpose: a Pool
  (gpsimd) op scales x_t[128p, 8q, 7f] by rT[:, :, t] (0-stride broadcast
  over f) and writes r itself into lane 7, then 8 PE transposes produce the
  [8, BL] fp16 moving operand.  All per-step DRAM broadcast DMAs are gone.
- fp16 everywhere on matmul operands and the elementwise chain: DVE runs in
  2x mode (594ns per [128,1024] op vs 1127 fp32); cell state c stays fp32.
- LN stats prepass runs directly in xq's batch-major [128, (q t)] layout
  via the quadratic-form identity (the stat scalars are per-sample
  constants, so no transposes are needed and rstd lands directly in rT's
  layout); stat constants are summed over h with an all-ones stationary
  matmul (no DRAM staging). y-chains on DVE, products/accumulation on Pool,
  with emission interleaved because cross-engine tile deps degrade to
  engine-counter joins.
- Layer 1 runs TWO timesteps behind layer 0: every ACT op in a steady-state
  period then depends only on >=half-period-old results, so the h0
  recurrence tail (tanh -> h-mult -> PE -> first gate ACT) hides entirely
  under L1's gate ops -- the ACT engine runs gap-free at its 10x1038ns/step
  floor.  All gate activations live in one ACT table (no table loads).

PSUM: "pg" gates 3x[128,1024] (6 banks), "px" x-transposes 2x[8,512]
(2 banks); prepass uses a separate pool that closes before the loop.
"""
import sys
sys.path.insert(0, '/opt/trn_rl_repo')

import numpy as np

B, T, F, H, D1, D2, OUT = 8192, 90, 7, 128, 128, 64, 30
NCORES = 8
BL = B // NCORES
G4 = 4 * H
NH = BL // 512
QB = BL // 128
EPS = 1e-5
C_F16 = True


def _build(nc, T_steps=T, dbg=False):
    import concourse.tile as tile
    from concourse import mybir
    from concourse.masks import make_identity

    f32 = mybir.dt.float32
    f16 = mybir.dt.float16
    AF = mybir.ActivationFunctionType
    ALU = mybir.AluOpType
    cdt = f16 if C_F16 else f32

    # ---------------- DRAM I/O ----------------
    x_d = nc.dram_tensor("x", [BL, T, F], f32, kind="ExternalInput")
    W_in_d = nc.dram_tensor("W_in", [H, F], f32, kind="ExternalInput")
    b_in_d = nc.dram_tensor("b_in", [H], f32, kind="ExternalInput")
    g_in_d = nc.dram_tensor("g_in", [H], f32, kind="ExternalInput")
    be_in_d = nc.dram_tensor("be_in", [H], f32, kind="ExternalInput")
    Wih_d = [nc.dram_tensor("Wih0", [G4, H], f32, kind="ExternalInput"),
             nc.dram_tensor("Wih1", [G4, H], f32, kind="ExternalInput")]
    Whh_d = [nc.dram_tensor("Whh0", [G4, H], f32, kind="ExternalInput"),
             nc.dram_tensor("Whh1", [G4, H], f32, kind="ExternalInput")]
    bih_d = [nc.dram_tensor("bih0", [G4], f32, kind="ExternalInput"),
             nc.dram_tensor("bih1", [G4], f32, kind="ExternalInput")]
    bhh_d = [nc.dram_tensor("bhh0", [G4], f32, kind="ExternalInput"),
             nc.dram_tensor("bhh1", [G4], f32, kind="ExternalInput")]
    g_ln_d = nc.dram_tensor("g_ln", [H], f32, kind="ExternalInput")
    be_ln_d = nc.dram_tensor("be_ln", [H], f32, kind="ExternalInput")
    W_d1_d = nc.dram_tensor("W_d1", [D1, H], f32, kind="ExternalInput")
    b_d1_d = nc.dram_tensor("b_d1", [D1], f32, kind="ExternalInput")
    W_d2_d = nc.dram_tensor("W_d2", [D2, D1], f32, kind="ExternalInput")
    b_d2_d = nc.dram_tensor("b_d2", [D2], f32, kind="ExternalInput")
    W_d3_d = nc.dram_tensor("W_d3", [OUT, D2], f32, kind="ExternalInput")
    b_d3_d = nc.dram_tensor("b_d3", [OUT], f32, kind="ExternalInput")
    out_d = nc.dram_tensor("out", [BL, OUT], f32, kind="ExternalOutput")
    if dbg:
        dbg_xt = nc.dram_tensor("dbg_xt", [8, BL], f32, kind="ExternalOutput")
        dbg_h0 = nc.dram_tensor("dbg_h0", [H, BL], f32, kind="ExternalOutput")
        dbg_c0 = nc.dram_tensor("dbg_c0", [H, BL], f32, kind="ExternalOutput")
        dbg_r = nc.dram_tensor("dbg_r", [T, BL], f32, kind="ExternalOutput")

    import contextlib
    with tile.TileContext(nc) as tc, contextlib.ExitStack() as ctx:
        singles = ctx.enter_context(tc.tile_pool(name="singles", bufs=1))
        trans = ctx.enter_context(tc.tile_pool(name="trans", bufs=2))
        small = ctx.enter_context(tc.tile_pool(name="small", bufs=2))
        xsp = ctx.enter_context(tc.tile_pool(name="xsp", bufs=3))
        dpool = ctx.enter_context(tc.tile_pool(name="dpool", bufs=1, space="DRAM"))

        # ---------------- constants ----------------
        ident = singles.tile([128, 128], f32)
        make_identity(nc, ident)
        ident16 = singles.tile([128, 128], f16)
        make_identity(nc, ident16)
        ones_row = singles.tile([1, 512], f32)
        nc.vector.memset(ones_row, 1.0)
        ones_col = singles.tile([128, 1], f32)
        nc.vector.memset(ones_col, 1.0)
        ones_col16 = singles.tile([128, 1], f16)
        nc.vector.memset(ones_col16, 1.0)
        ones_row90 = singles.tile([1, T], f32)
        nc.vector.memset(ones_row90, 1.0)
        ones_row128_16 = singles.tile([1, 128], f16)
        nc.vector.memset(ones_row128_16, 1.0)
        eps_col = singles.tile([128, 1], f32)
        nc.vector.memset(eps_col, EPS)
        ones_q = singles.tile([128, QB], f32)
        nc.vector.memset(ones_q, 1.0)

        def load_col(dram_vec, n, name):
            t_ = singles.tile([n, 1], f32, name=name, tag=name)
            nc.sync.dma_start(out=t_, in_=dram_vec[:].rearrange("(p o) -> p o", o=1))
            return t_

        w_in_raw = singles.tile([H, F], f32)
        nc.sync.dma_start(out=w_in_raw, in_=W_in_d[:, :])
        b_in_c = load_col(b_in_d, H, "b_in_c")
        g_in_c = load_col(g_in_d, H, "g_in_c")
        be_in_c = load_col(be_in_d, H, "be_in_c")
        g_ln_c = load_col(g_ln_d, H, "g_ln_c")
        be_ln_c = load_col(be_ln_d, H, "be_ln_c")
        b_d1_c = load_col(b_d1_d, D1, "b_d1_c")
        b_d2_c = load_col(b_d2_d, D2, "b_d2_c")
        b_d3_c = load_col(b_d3_d, OUT, "b_d3_c")

        # ---------------- x loads ----------------
        # xq[p, q, t, f] = x[128q+p, t, f]  (contiguous 2520B runs per (p,q))
        xq = singles.tile([128, QB, T, F], f32)
        nc.sync.dma_start(
            out=xq, in_=x_d[:, :, :].rearrange("(q p) t f -> p q t f", p=128))
        xqh = singles.tile([128, QB, T, F], f16)

        # ------- weights: load + PE-transpose; LN fold into layer-0 -------
        with tc.tile_pool(name="ps_pre", bufs=3, space="PSUM") as pre:
            def transpose_to(dst, src_ap, p, fdim):
                pt = pre.tile([fdim, p], f32, tag="scr", name="tr_ps")
                nc.tensor.transpose(pt, src_ap, ident[:p, :p])
                nc.vector.tensor_copy(out=dst, in_=pt)

            # stat constants, all-partition broadcast WITHOUT a DRAM
            # roundtrip: rhs columns hold per-h products; contracting with an
            # all-ones [128,128] stationary sums over h into every partition.
            NST = F * F + (F + 2) + (F + 1)
            rhs_all = small.tile([H, NST], f32, tag="rhs_all", name="rhs_all")
            for i in range(F):
                nc.vector.tensor_tensor(
                    out=rhs_all[:, i * F:(i + 1) * F], in0=w_in_raw,
                    in1=w_in_raw[:, i:i + 1].to_broadcast([H, F]), op=ALU.mult)
            nc.vector.tensor_scalar_mul(out=rhs_all[:, F * F:F * F + F],
                                        in0=w_in_raw, scalar1=b_in_c)
            nc.vector.tensor_tensor(out=rhs_all[:, F * F + F:F * F + F + 1],
                                    in0=b_in_c, in1=b_in_c, op=ALU.mult)
            nc.vector.tensor_scalar_mul(
                out=rhs_all[:, F * F + F + 1:F * F + F + 2], in0=b_in_c,
                scalar1=1.0)
            nc.vector.tensor_scalar_mul(
                out=rhs_all[:, F * F + F + 2:F * F + F + 2 + F], in0=w_in_raw,
                scalar1=1.0 / H)
            nc.vector.tensor_scalar_mul(out=rhs_all[:, NST - 1:NST],
                                        in0=b_in_c, scalar1=1.0 / H)
            ones128 = singles.tile([128, 128], f32)
            nc.vector.memset(ones128, 1.0)
            sbc_ps = pre.tile([128, NST], f32, tag="sbc", name="sbc_ps",
                              bufs=1)
            nc.tensor.matmul(sbc_ps, ones128, rhs_all, start=True, stop=True)
            # p_ws/wsn (partition-0 row) still needed for the LN weight fold
            p_ws = pre.tile([1, F + 1], f32, tag="scr", name="p_ws")
            nc.tensor.matmul(p_ws[:, 0:F], ones_col, w_in_raw, start=True,
                             stop=False, skip_group_check=True)
            nc.tensor.matmul(p_ws[:, F:F + 1], ones_col, b_in_c, start=False,
                             stop=True, skip_group_check=True)
            wsn = small.tile([1, F + 1], f32, tag="wsn", name="wsn")
            nc.vector.tensor_scalar_mul(out=wsn, in0=p_ws, scalar1=1.0 / H)
            sbc = singles.tile([128, NST], f32)
            nc.vector.tensor_copy(out=sbc, in_=sbc_ps)

            wihT0f = singles.tile([H, 4, H], f32)  # raw Wih0^T per gate
            wihT1 = singles.tile([H, 4, H], f16)
            whhT = [singles.tile([H, 4, H], f16, name=f"whhT{L}", tag=f"whhT{L}")
                    for L in range(2)]
            for L in range(2):
                for cc in range(4):
                    raw = trans.tile([H, H], f32, tag="u", name="raw")
                    nc.sync.dma_start(out=raw, in_=Wih_d[L][cc * H:(cc + 1) * H, :])
                    pt_w = pre.tile([H, H], f32, tag="scr", name="tr_ps_w")
                    nc.tensor.transpose(pt_w, raw, ident)
                    if L == 0:
                        nc.vector.tensor_copy(out=wihT0f[:, cc, :], in_=pt_w)
                    else:
                        nc.vector.tensor_copy(out=wihT1[:, cc, :], in_=pt_w)
                    raw2 = trans.tile([H, H], f32, tag="v_", name="raw2")
                    nc.sync.dma_start(out=raw2, in_=Whh_d[L][cc * H:(cc + 1) * H, :])
                    transpose_to(whhT[L][:, cc, :], raw2, H, H)

            # gate biases beff[L] [128, 4]; layer-0 gains Wih0 @ be_in
            beff = []
            for L in range(2):
                bt_ = singles.tile([H, 4], f32, name=f"beff{L}", tag=f"beff{L}")
                bih_sb = small.tile([H, 4], f32, tag="bload", name="bih_sb")
                nc.sync.dma_start(out=bih_sb,
                                  in_=bih_d[L][:].rearrange("(c p) -> p c", p=H))
                bhh_sb = small.tile([H, 4], f32, tag="bload2", name="bhh_sb")
                nc.sync.dma_start(out=bhh_sb,
                                  in_=bhh_d[L][:].rearrange("(c p) -> p c", p=H))
                nc.vector.tensor_add(out=bt_, in0=bih_sb, in1=bhh_sb)
                beff.append(bt_)
            for cc in range(4):
                pb = pre.tile([H, 1], f32, tag="scr", name="pb")
                nc.tensor.matmul(pb, wihT0f[:, cc, :], be_in_c, start=True,
                                 stop=True)
                nc.vector.tensor_add(out=beff[0][:, cc:cc + 1],
                                     in0=beff[0][:, cc:cc + 1], in1=pb)

            # ---- LN fold: Pg = diag(g_in) [W_in - 1 ws^T/H | b_in - bs/H] ----
            pw_bc = pre.tile([H, F + 1], f32, tag="scr", name="pw_bc")
            nc.tensor.matmul(pw_bc, ones_row[:, 0:H], wsn, start=True, stop=True)
            cat8 = small.tile([H, F + 1], f32, tag="cat8", name="cat8")
            nc.vector.tensor_copy(out=cat8[:, 0:F], in_=w_in_raw)
            nc.vector.tensor_copy(out=cat8[:, F:F + 1], in_=b_in_c)
            Pg = singles.tile([H, F + 1], f32)
            nc.vector.tensor_sub(out=Pg, in0=cat8, in1=pw_bc)
            nc.vector.tensor_scalar_mul(out=Pg, in0=Pg, scalar1=g_in_c)
            stat0 = singles.tile([F + 1, 4, H], f16)
            for cc in range(4):
                ps8 = pre.tile([F + 1, H], f32, tag="scr", name="ps8")
                nc.tensor.matmul(ps8, Pg, wihT0f[:, cc, :], start=True, stop=True)
                nc.vector.tensor_copy(out=stat0[:, cc, :], in_=ps8)

            # dense head weights (transposed, f16 stationaries)
            wd1T = singles.tile([H, D1], f16)
            wd1_raw = trans.tile([D1, H], f32, tag="u", name="wd1_raw")
            nc.sync.dma_start(out=wd1_raw, in_=W_d1_d[:, :])
            transpose_to(wd1T, wd1_raw, D1, H)
            wd2T = singles.tile([D1, D2], f16)
            wd2_raw = trans.tile([D2, D1], f32, tag="v_", name="wd2_raw")
            nc.sync.dma_start(out=wd2_raw, in_=W_d2_d[:, :])
            transpose_to(wd2T, wd2_raw, D2, D1)
            wd3T = singles.tile([D2, OUT], f16)
            wd3_raw = trans.tile([OUT, D2], f32, tag="u", name="wd3_raw")
            nc.sync.dma_start(out=wd3_raw, in_=W_d3_d[:, :])
            transpose_to(wd3T, wd3_raw, OUT, D2)

            # -------- prepass: LN stats in batch-major [128, (q t)] --------
            # per (t,b) sample:  sum_h p = ws.x + bs ;
            #   sum_h p^2 = x^T M x + 2 l^T x + c0  (M = W^T W, l = W^T b).
            # The stat scalars are per-sample CONSTANTS, so the stats run
            # directly on xq's own [128p, (q t)] layout -- no transposes --
            # and rstd lands directly in rT's batch-major layout.
            mbc = sbc[:, 0:F * F]
            lbc = sbc[:, F * F:F * F + F]
            c0bc = sbc[:, F * F + F:F * F + F + 1]
            wbc = sbc[:, F * F + F + 2:F * F + F + 2 + F]  # ws/H
            bshbc = sbc[:, NST - 1:NST]  # bs/H

            TS = T_steps

            def xf(fi):
                return xq[:, :, :TS, fi]

            rT = singles.tile([128, QB, T], f32)
            nmu = trans.tile([128, QB, T], f32, tag="sig_i", name="st_nmu")
            nc.vector.tensor_scalar_mul(out=nmu[:, :, :TS], in0=xf(0),
                                        scalar1=wbc[:, 0:1])
            for fi in range(1, F):
                nc.vector.scalar_tensor_tensor(
                    out=nmu[:, :, :TS], in0=xf(fi), scalar=wbc[:, fi:fi + 1],
                    in1=nmu[:, :, :TS], op0=ALU.mult, op1=ALU.add)
            # nmu = -(ws.x/H + bs/H)
            nc.vector.tensor_scalar(out=nmu[:, :, :TS], in0=nmu[:, :, :TS],
                                    scalar1=bshbc, scalar2=-1.0,
                                    op0=ALU.add, op1=ALU.mult)
            # quadratic form: y-chains (TensorScalarPtr, DVE-only) on DVE;
            # the x*y products and qacc accumulation (plain TensorTensor) on
            # Pool. Emission inline keeps the conservative cross-engine sem
            # joins tight.
            qacc = trans.tile([128, QB, T], f32, tag="x_pool_a", name="st_qacc")
            tprod = trans.tile([128, QB, T], f32, tag="x_pool_b",
                               name="st_tprod")
            for fi in range(F):
                yf = trans.tile([128, QB, T], f32, tag="st_yf", name="st_yf")
                nc.vector.tensor_scalar_mul(out=yf[:, :, :TS], in0=xf(0),
                                            scalar1=mbc[:, fi * F:fi * F + 1])
                for fj in range(1, F):
                    nc.vector.scalar_tensor_tensor(
                        out=yf[:, :, :TS], in0=xf(fj),
                        scalar=mbc[:, fi * F + fj:fi * F + fj + 1],
                        in1=yf[:, :, :TS], op0=ALU.mult, op1=ALU.add)
                if fi == 0:
                    nc.gpsimd.tensor_tensor(out=qacc[:, :, :TS], in0=xf(fi),
                                            in1=yf[:, :, :TS], op=ALU.mult)
                else:
                    nc.gpsimd.tensor_tensor(out=tprod[:, :, :TS], in0=xf(fi),
                                            in1=yf[:, :, :TS], op=ALU.mult)
                    nc.gpsimd.tensor_add(out=qacc[:, :, :TS],
                                         in0=qacc[:, :, :TS],
                                         in1=tprod[:, :, :TS])
            # + 2 l.x
            lin = trans.tile([128, QB, T], f32, tag="st_lin", name="st_lin")
            nc.vector.tensor_scalar_mul(out=lin[:, :, :TS], in0=xf(0),
                                        scalar1=lbc[:, 0:1])
            for fi in range(1, F):
                nc.vector.scalar_tensor_tensor(
                    out=lin[:, :, :TS], in0=xf(fi), scalar=lbc[:, fi:fi + 1],
                    in1=lin[:, :, :TS], op0=ALU.mult, op1=ALU.add)
            nc.vector.scalar_tensor_tensor(out=qacc[:, :, :TS],
                                           in0=lin[:, :, :TS],
                                           scalar=2.0, in1=qacc[:, :, :TS],
                                           op0=ALU.mult, op1=ALU.add)
            # var = (q + c0)/H - mu^2 ; rT = 1/sqrt(var+eps)
            nc.vector.tensor_scalar(out=qacc[:, :, :TS], in0=qacc[:, :, :TS],
                                    scalar1=c0bc, scalar2=1.0 / H,
                                    op0=ALU.add, op1=ALU.mult)
            musq = trans.tile([128, QB, T], f32, tag="st_yf", name="st_musq")
            nc.vector.tensor_tensor(out=musq[:, :, :TS], in0=nmu[:, :, :TS],
                                    in1=nmu[:, :, :TS], op=ALU.mult)
            nc.vector.tensor_sub(out=qacc[:, :, :TS], in0=qacc[:, :, :TS],
                                 in1=musq[:, :, :TS])
            nc.scalar.activation(out=rT[:, :, :TS], in_=qacc[:, :, :TS],
                                 func=AF.Sqrt, bias=eps_col, scale=1.0)
            nc.vector.reciprocal(out=rT[:, :, :TS], in_=rT[:, :, :TS])
            # f32 -> f16 x copy for the loop (Pool; runs during the DVE tail)
            nc.gpsimd.tensor_tensor(
                out=xqh[:, :, :, :].rearrange("p q t f -> p (q t f)"),
                in0=xq[:, :, :, :].rearrange("p q t f -> p (q t f)"),
                in1=ones_col[:, 0:1].to_broadcast([128, QB * T * F]),
                op=ALU.mult)

        # ---------------- states ----------------
        h1 = singles.tile([H, BL], f16, name="h1", tag="h1")
        c = [singles.tile([H, BL], cdt, name="c0", tag="c0"),
             singles.tile([H, BL], cdt, name="c1", tag="c1")]
        h0_z = trans.tile([H, BL], f16, tag="h0", name="h0_init", bufs=3)
        nc.vector.memset(h0_z, 0.0)
        nc.vector.memset(h1, 0.0)
        for L in range(2):
            nc.vector.memset(c[L], 0.0)
        # layer-1 runs TWO steps behind layer-0: every ACT op in a period then
        # depends only on >= half-period-old results, so the h0 recurrence
        # tail (tanh -> h-mult -> PE -> first gate ACT) hides under L1's ops.
        h0_hist = [None, h0_z]

        ps_pg = ctx.enter_context(tc.tile_pool(name="ps_pg", bufs=3, space="PSUM"))
        ps_px = ctx.enter_context(tc.tile_pool(name="ps_px", bufs=2, space="PSUM"))

        def pg_tile(shape, name):
            return ps_pg.tile(shape, f32, tag="pg", name=name)

        # ---------------- main loop ----------------
        def lstm_step(L, inp, hprev, hout, hh_first, split=False,
                      first_gate_split=False):
            sig_i = trans.tile([H, BL], f16, tag="sig_i", name="sig_i")
            sig_f = trans.tile([H, BL], f16, tag="sig_f", name="sig_f")
            tg = trans.tile([H, BL], f16, tag="tg", name="tg")
            sig_o = trans.tile([H, BL], f16, tag="sig_o", name="sig_o")
            outs = [sig_i, sig_f, tg, sig_o]
            funcs = [AF.Sigmoid, AF.Sigmoid, AF.Tanh, AF.Sigmoid]
            wih = stat0 if L == 0 else wihT1
            for gc in range(4):
                pg = pg_tile([H, BL], "pg_gates")
                for hc in range(NH):
                    sl = slice(hc * 512, (hc + 1) * 512)
                    ops = [(wih[:, gc, :], inp), (whhT[L][:, gc, :], hprev)]
                    if hh_first:
                        ops.reverse()
                    nc.tensor.matmul(pg[:, sl], ops[0][0], ops[0][1][:, sl],
                                     start=True, stop=False)
                    nc.tensor.matmul(pg[:, sl], ops[1][0], ops[1][1][:, sl],
                                     start=False, stop=True)
                if gc == 0 and first_gate_split:
                    for hc in range(NH):
                        sl = slice(hc * 512, (hc + 1) * 512)
                        nc.scalar.activation(out=outs[gc][:, sl],
                                             in_=pg[:, sl], func=funcs[gc],
                                             bias=beff[L][:, gc:gc + 1],
                                             scale=1.0)
                else:
                    nc.scalar.activation(out=outs[gc], in_=pg, func=funcs[gc],
                                         bias=beff[L][:, gc:gc + 1], scale=1.0)
            u = trans.tile([H, BL], f16, tag="u", name="u")
            v_ = trans.tile([H, BL], cdt, tag="v_", name="v_")
            tc_ = trans.tile([H, BL], f16, tag="tc_", name="tc_")
            if not split:
                nc.vector.tensor_tensor(out=v_, in0=sig_f, in1=c[L], op=ALU.mult)
                nc.vector.tensor_tensor(out=u, in0=sig_i, in1=tg, op=ALU.mult)
                nc.vector.tensor_add(out=c[L], in0=u, in1=v_)
                nc.scalar.activation(out=tc_, in_=c[L], func=AF.Tanh, scale=1.0)
                nc.vector.tensor_tensor(out=hout, in0=sig_o, in1=tc_, op=ALU.mult)
            else:
                # half-column tail: lets tanh/h pipeline against the DVE chain
                # v-products first: they only need sig_f (2nd ACT op)
                for hc in range(NH):
                    sl = slice(hc * 512, (hc + 1) * 512)
                    nc.vector.tensor_tensor(out=v_[:, sl], in0=sig_f[:, sl],
                                            in1=c[L][:, sl], op=ALU.mult)
                for hc in range(NH):
                    sl = slice(hc * 512, (hc + 1) * 512)
                    nc.vector.tensor_tensor(out=u[:, sl], in0=sig_i[:, sl],
                                            in1=tg[:, sl], op=ALU.mult)
                    nc.vector.tensor_add(out=c[L][:, sl], in0=u[:, sl],
                                         in1=v_[:, sl])
                for hc in range(NH):
                    sl = slice(hc * 512, (hc + 1) * 512)
                    nc.scalar.activation(out=tc_[:, sl], in_=c[L][:, sl],
                                         func=AF.Tanh, scale=1.0)
                for hc in range(NH):
                    sl = slice(hc * 512, (hc + 1) * 512)
                    nc.vector.tensor_tensor(out=hout[:, sl], in0=sig_o[:, sl],
                                            in1=tc_[:, sl], op=ALU.mult)

        for t in range(T_steps):
            # scale x_t by rstd in batch-major layout (Pool), lane 7 = rstd
            xs = xsp.tile([128, QB, F + 1], f16, tag="xs", name="xs")
            nc.gpsimd.tensor_tensor(
                out=xs[:, :, 0:F], in0=xqh[:, :, t, :],
                in1=rT[:, :, t:t + 1].to_broadcast([128, QB, F]), op=ALU.mult)
            nc.gpsimd.tensor_tensor(out=xs[:, :, F], in0=rT[:, :, t],
                                    in1=ones_q, op=ALU.mult)
            # PE transpose to [8, BL] fp16 moving operand xt = [x r ; r]
            xt = trans.tile([F + 1, BL], f16, tag="xt", name="xt")
            for half in range(2):
                px = ps_px.tile([F + 1, 512], f16, tag="pxt", name="pxt")
                for qi in range(4):
                    q = half * 4 + qi
                    nc.tensor.transpose(px[:, qi * 128:(qi + 1) * 128],
                                        xs[:, q, :], ident16)
                nc.vector.tensor_copy(
                    out=xt[:, half * 512:(half + 1) * 512], in_=px)
            if t > 1:
                lstm_step(1, h0_hist[0], h1, h1, hh_first=True)
            h0_new = trans.tile([H, BL], f16, tag="h0", name="h0_new", bufs=3)
            lstm_step(0, xt, h0_hist[1], h0_new, hh_first=False, split=False)
            h0_hist = [h0_hist[1], h0_new]
            if dbg and t == 0:
                xtc = trans.tile([F + 1, BL], f32, tag="v_", name="xtc_dbg")
                nc.vector.tensor_copy(out=xtc, in_=xt)
                nc.sync.dma_start(out=dbg_xt[:, :], in_=xtc)
                h0c = trans.tile([H, BL], f32, tag="u", name="h0c_dbg")
                nc.vector.tensor_copy(out=h0c, in_=h0_new)
                nc.sync.dma_start(out=dbg_h0[:, :], in_=h0c)
                c0c = trans.tile([H, BL], f32, tag="tc_", name="c0c_dbg")
                nc.vector.tensor_copy(out=c0c, in_=c[0])
                nc.sync.dma_start(out=dbg_c0[:, :], in_=c0c)
        lstm_step(1, h0_hist[0], h1, h1, hh_first=True, split=True,
                  first_gate_split=True)
        lstm_step(1, h0_hist[1], h1, h1, hh_first=True, split=True,
                  first_gate_split=True)

        # ---------------- head ----------------
        sqh = trans.tile([H, BL], f16, tag="sig_f", name="sqh")
        nc.vector.tensor_tensor(out=sqh, in0=h1, in1=h1, op=ALU.mult)
        ps_s1 = pg_tile([1, BL], "ps_s1")
        ps_s2 = pg_tile([1, BL], "ps_s2")
        for hc in range(NH):
            sl = slice(hc * 512, (hc + 1) * 512)
            nc.tensor.matmul(ps_s1[:, sl], ones_col16, h1[:, sl],
                             start=True, stop=True, skip_group_check=True)
            nc.tensor.matmul(ps_s2[:, sl], ones_col16, sqh[:, sl],
                             start=True, stop=True, skip_group_check=True)
        # head LN stats: scale ops on ACT (f16 out), multiplies on DVE at 2x
        nm16 = singles.tile([1, BL], f16, tag="nm16", name="nm16")
        nc.scalar.activation(out=nm16, in_=ps_s1, func=AF.Copy,
                             scale=-1.0 / H)
        v16 = singles.tile([1, BL], f16, tag="v16", name="v16")
        nc.scalar.activation(out=v16, in_=ps_s2, func=AF.Copy, scale=1.0 / H)
        musq_h = singles.tile([1, BL], f16, tag="musq", name="musq_h")
        nc.vector.tensor_tensor(out=musq_h, in0=nm16, in1=nm16, op=ALU.mult)
        with nc.allow_low_precision(reason="head LN var in f16; |var|~O(1)"):
            nc.vector.tensor_sub(out=v16, in0=v16, in1=musq_h)
        rh16 = singles.tile([1, BL], f16, tag="rh16", name="rh16")
        nc.scalar.activation(out=rh16, in_=v16, func=AF.Sqrt,
                             bias=eps_col[0:1], scale=1.0)
        with nc.allow_low_precision(reason="head LN rstd in f16"):
            nc.vector.reciprocal(out=rh16, in_=rh16)
        pnm = pg_tile([H, BL], "pnm")
        prh = ps_px.tile([H, 512], f32, tag="pxt", name="prh0")
        prh2 = ps_px.tile([H, 512], f32, tag="pxt", name="prh1")
        prhs = [prh, prh2]
        for hc in range(NH):
            sl = slice(hc * 512, (hc + 1) * 512)
            nc.tensor.matmul(pnm[:, sl], ones_row128_16, nm16[:, sl],
                             start=True, stop=True, skip_group_check=True)
            nc.tensor.matmul(prhs[hc], ones_row128_16, rh16[:, sl],
                             start=True, stop=True, skip_group_check=True)
        t1 = trans.tile([H, BL], f32, tag="tg", name="t1")
        nc.vector.tensor_tensor(out=t1, in0=h1, in1=pnm, op=ALU.add)
        t2 = trans.tile([H, BL], f32, tag="sig_o", name="t2")
        for hc in range(NH):
            sl = slice(hc * 512, (hc + 1) * 512)
            nc.vector.tensor_tensor(out=t2[:, sl], in0=t1[:, sl], in1=prhs[hc],
                                    op=ALU.mult)
        last = trans.tile([H, BL], f16, tag="u", name="last")
        nc.vector.tensor_scalar(out=last, in0=t2, scalar1=g_ln_c,
                                scalar2=be_ln_c, op0=ALU.mult, op1=ALU.add)
        pd1 = pg_tile([D1, BL], "pd1")
        for hc in range(NH):
            sl = slice(hc * 512, (hc + 1) * 512)
            nc.tensor.matmul(pd1[:, sl], wd1T, last[:, sl], start=True, stop=True,
                             skip_group_check=True)
        d1 = trans.tile([D1, BL], f16, tag="v_", name="d1")
        nc.scalar.activation(out=d1, in_=pd1, func=AF.Relu, bias=b_d1_c, scale=1.0)
        pd2 = pg_tile([D2, BL], "pd2")
        for hc in range(NH):
            sl = slice(hc * 512, (hc + 1) * 512)
            nc.tensor.matmul(pd2[:, sl], wd2T, d1[:, sl], start=True, stop=True,
                             skip_group_check=True)
        d2 = trans.tile([D2, BL], f16, tag="tc_", name="d2")
        nc.scalar.activation(out=d2, in_=pd2, func=AF.Relu, bias=b_d2_c, scale=1.0)
        pd3 = pg_tile([OUT, BL], "pd3")
        for hc in range(NH):
            sl = slice(hc * 512, (hc + 1) * 512)
            nc.tensor.matmul(pd3[:, sl], wd3T, d2[:, sl], start=True, stop=True,
                             skip_group_check=True)
        o3 = trans.tile([OUT, BL], f32, tag="sig_f", name="o3")
        nc.scalar.activation(out=o3, in_=pd3, func=AF.Identity, bias=b_d3_c,
                             scale=1.0)
        outT = singles.tile([128, QB, OUT], f32)
        for q in range(QB):
            pot = ps_px.tile([128, OUT], f32, tag="pxt", name="pot")
            nc.tensor.transpose(pot, o3[:, q * 128:(q + 1) * 128],
                                ident[:OUT, :OUT])
            nc.vector.tensor_copy(out=outT[:, q, :], in_=pot)
        nc.sync.dma_start(
            out=out_d[:, :].rearrange("(q p) c -> p q c", p=128),
            in_=outT)
    return nc


_CACHE = {}


def _get_runner(T_steps=T):
    if "runner" in _CACHE:
        return _CACHE["runner"]
    import jax
    from jax.sharding import Mesh, PartitionSpec
    from jax.experimental.shard_map import shard_map
    import concourse.bacc as bacc
    import concourse.mybir as mybir
    from concourse.bass2jax import install_neuronx_cc_hook, _bass_exec_p, \
        partition_id_tensor

    nc = bacc.Bacc()
    _build(nc, T_steps=T_steps)
    nc.compile()
    install_neuronx_cc_hook()

    partition_name = nc.partition_id_tensor.name if nc.partition_id_tensor else None
    in_names, out_names, out_avals, zero_outs = [], [], [], []
    for alloc in nc.m.functions[0].allocations:
        if not isinstance(alloc, mybir.MemoryLocationSet):
            continue
        name = alloc.memorylocations[0].name
        if alloc.kind == "ExternalInput":
            if name != partition_name:
                in_names.append(name)
        elif alloc.kind == "ExternalOutput":
            out_names.append(name)
            shape = tuple(alloc.tensor_shape)
            dtype = mybir.dt.np(alloc.dtype)
            out_avals.append(jax.core.ShapedArray(shape, dtype))
            zero_outs.append(np.zeros(shape, dtype))
    n_params = len(in_names)
    all_in_names = in_names + out_names + ([partition_name] if partition_name else [])

    def _body(*args):
        operands = list(args)
        if partition_name is not None:
            operands.append(partition_id_tensor())
        outs = _bass_exec_p.bind(
            *operands,
            out_avals=tuple(out_avals),
            in_names=tuple(all_in_names),
            out_names=tuple(out_names),
            lowering_input_output_aliases=(),
            sim_require_finite=False,
            sim_require_nnan=False,
            nc=nc,
        )
        return tuple(outs)

    devices = jax.devices()[:NCORES]
    mesh = Mesh(np.asarray(devices), ("core",))
    in_specs = (PartitionSpec("core"),) * (n_params + len(out_names))
    out_specs = (PartitionSpec("core"),) * len(out_names)
    sharded = jax.jit(
        shard_map(_body, mesh=mesh, in_specs=in_specs, out_specs=out_specs,
                  check_rep=False),
        keep_unused=True)
    _CACHE["runner"] = (sharded, in_names, out_names, zero_outs)
    return _CACHE["runner"]


def kernel(**inputs) -> np.ndarray:
    sharded, in_names, out_names, zero_outs = _get_runner()
    inp = {k: np.ascontiguousarray(np.asarray(v), dtype=np.float32)
           for k, v in inputs.items()}

    def core_val(name, ci):
        if name == "x":
            return inp["x"][ci * BL:(ci + 1) * BL]
        return inp[name]

    concat_in = [
        np.concatenate([core_val(n, ci) for ci in range(NCORES)], axis=0)
        for n in in_names
    ]
    concat_zeros = [
        np.zeros((NCORES * z.shape[0], *z.shape[1:]), z.dtype) for z in zero_outs
    ]
    import jax
    out_arrs = sharded(*concat_in, *concat_zeros)
    jax.block_until_ready(out_arrs)
    oi = out_names.index("out")
    full = np.asarray(out_arrs[oi]).reshape(B, OUT)
    return full.astype(np.float32)


# revision 36
# speedup vs baseline: 1.0550x; 1.0012x over previous
"""DepletionLSTM Trainium2 kernel (v2).

Self-contained: builds a Bass/Tile kernel for the 2-layer-LSTM network,
shards the batch over 8 NeuronCores (pure data parallelism), runs via
PJRT/axon, returns the full [8192, 30] float32 output.

Strategy (per core, 1024 batch):
- All activations SBUF-resident; zero in-loop DRAM traffic.
- The input-projection LayerNorm is folded INTO the layer-0 gate weights:
  x0 = (W_in x + b_in - mu 1) r  ==  [W'|b'] @ [x r ; r]  with
  W' = W_in - 1 ws^T/H, b' = b_in - bs/H, so the layer-0 input-gate matmul
  uses an 8-row stationary Stat0 = ([W'|b'])^T diag(g_in) Wih0^T and the
  8-row moving operand xt = [x r ; r].  No separate projection matmul, no
  x0 tile, no PSUM->SBUF projection copy.
- rstd (r) is applied in batch-major layout BEFORE the PE transpose: a Pool
  (gpsimd) op scales x_t[128p, 8q, 7f] by rT[:, :, t] (0-stride broadcast
  over f) and writes r itself into lane 7, then 8 PE transposes produce the
  [8, BL] fp16 moving operand.  All per-step DRAM broadcast DMAs are gone.
- fp16 everywhere on matmul operands and the elementwise chain: DVE runs in
  2x mode (594ns per [128,1024] op vs 1127 fp32); cell state c stays fp32.
- LN stats prepass runs directly in xq's batch-major [128, (q t)] layout
  via the quadratic-form identity (the stat scalars are per-sample
  constants, so no transposes are needed and rstd lands directly in rT's
  layout); stat constants are summed over h with an all-ones stationary
  matmul (no DRAM staging). y-chains on DVE, products/accumulation on Pool,
  with emission interleaved because cross-engine tile deps degrade to
  engine-counter joins.
- Layer 1 runs TWO timesteps behind layer 0: every ACT op in a steady-state
  period then depends only on >=half-period-old results, so the h0
  recurrence tail (tanh -> h-mult -> PE -> first gate ACT) hides entirely
  under L1's gate ops -- the ACT engine runs gap-free at its 10x1038ns/step
  floor.  All gate activations live in one ACT table (no table loads).

PSUM: "pg" gates 3x[128,1024] (6 banks), "px" x-transposes 2x[8,512]
(2 banks); prepass uses a separate pool that closes before the loop.
"""
import sys
sys.path.insert(0, '/opt/trn_rl_repo')

import numpy as np

B, T, F, H, D1, D2, OUT = 8192, 90, 7, 128, 128, 64, 30
NCORES = 8
BL = B // NCORES
G4 = 4 * H
NH = BL // 512
QB = BL // 128
EPS = 1e-5
C_F16 = True


def _build(nc, T_steps=T, dbg=False):
    import concourse.tile as tile
    from concourse import mybir
    from concourse.masks import make_identity

    f32 = mybir.dt.float32
    f16 = mybir.dt.float16
    AF = mybir.ActivationFunctionType
    ALU = mybir.AluOpType
    cdt = f16 if C_F16 else f32

    # ---------------- DRAM I/O ----------------
    x_d = nc.dram_tensor("x", [BL, T, F], f32, kind="ExternalInput")
    W_in_d = nc.dram_tensor("W_in", [H, F], f32, kind="ExternalInput")
    b_in_d = nc.dram_tensor("b_in", [H], f32, kind="ExternalInput")
    g_in_d = nc.dram_tensor("g_in", [H], f32, kind="ExternalInput")
    be_in_d = nc.dram_tensor("be_in", [H], f32, kind="ExternalInput")
    Wih_d = [nc.dram_tensor("Wih0", [G4, H], f32, kind="ExternalInput"),
             nc.dram_tensor("Wih1", [G4, H], f32, kind="ExternalInput")]
    Whh_d = [nc.dram_tensor("Whh0", [G4, H], f32, kind="ExternalInput"),
             nc.dram_tensor("Whh1", [G4, H], f32, kind="ExternalInput")]
    bih_d = [nc.dram_tensor("bih0", [G4], f32, kind="ExternalInput"),
             nc.dram_tensor("bih1", [G4], f32, kind="ExternalInput")]
    bhh_d = [nc.dram_tensor("bhh0", [G4], f32, kind="ExternalInput"),
             nc.dram_tensor("bhh1", [G4], f32, kind="ExternalInput")]
    g_ln_d = nc.dram_tensor("g_ln", [H], f32, kind="ExternalInput")
    be_ln_d = nc.dram_tensor("be_ln", [H], f32, kind="ExternalInput")
    W_d1_d = nc.dram_tensor("W_d1", [D1, H], f32, kind="ExternalInput")
    b_d1_d = nc.dram_tensor("b_d1", [D1], f32, kind="ExternalInput")
    W_d2_d = nc.dram_tensor("W_d2", [D2, D1], f32, kind="ExternalInput")
    b_d2_d = nc.dram_tensor("b_d2", [D2], f32, kind="ExternalInput")
    W_d3_d = nc.dram_tensor("W_d3", [OUT, D2], f32, kind="ExternalInput")
    b_d3_d = nc.dram_tensor("b_d3", [OUT], f32, kind="ExternalInput")
    out_d = nc.dram_tensor("out", [BL, OUT], f32, kind="ExternalOutput")
    if dbg:
        dbg_xt = nc.dram_tensor("dbg_xt", [8, BL], f32, kind="ExternalOutput")
        dbg_h0 = nc.dram_tensor("dbg_h0", [H, BL], f32, kind="ExternalOutput")
        dbg_c0 = nc.dram_tensor("dbg_c0", [H, BL], f32, kind="ExternalOutput")
        dbg_r = nc.dram_tensor("dbg_r", [T, BL], f32, kind="ExternalOutput")

    import contextlib
    with tile.TileContext(nc) as tc, contextlib.ExitStack() as ctx:
        singles = ctx.enter_context(tc.tile_pool(name="singles", bufs=1))
        trans = ctx.enter_context(tc.tile_pool(name="trans", bufs=2))
        small = ctx.enter_context(tc.tile_pool(name="small", bufs=2))
        xsp = ctx.enter_context(tc.tile_pool(name="xsp", bufs=3))
        dpool = ctx.enter_context(tc.tile_pool(name="dpool", bufs=1, space="DRAM"))

        # ---------------- constants ----------------
        ident = singles.tile([128, 128], f32)
        make_identity(nc, ident)
        ident16 = singles.tile([128, 128], f16)
        make_identity(nc, ident16)
        ones_row = singles.tile([1, 512], f32)
        nc.vector.memset(ones_row, 1.0)
        ones_col = singles.tile([128, 1], f32)
        nc.vector.memset(ones_col, 1.0)
        ones_col16 = singles.tile([128, 1], f16)
        nc.vector.memset(ones_col16, 1.0)
        ones_row90 = singles.tile([1, T], f32)
        nc.vector.memset(ones_row90, 1.0)
        ones_row128_16 = singles.tile([1, 128], f16)
        nc.vector.memset(ones_row128_16, 1.0)
        eps_col = singles.tile([128, 1], f32)
        nc.vector.memset(eps_col, EPS)
        ones_q = singles.tile([128, QB], f32)
        nc.vector.memset(ones_q, 1.0)

        def load_col(dram_vec, n, name):
            t_ = singles.tile([n, 1], f32, name=name, tag=name)
            nc.sync.dma_start(out=t_, in_=dram_vec[:].rearrange("(p o) -> p o", o=1))
            return t_

        w_in_raw = singles.tile([H, F], f32)
        nc.sync.dma_start(out=w_in_raw, in_=W_in_d[:, :])
        b_in_c = load_col(b_in_d, H, "b_in_c")
        g_in_c = load_col(g_in_d, H, "g_in_c")
        be_in_c = load_col(be_in_d, H, "be_in_c")
        g_ln_c = load_col(g_ln_d, H, "g_ln_c")
        be_ln_c = load_col(be_ln_d, H, "be_ln_c")
        b_d1_c = load_col(b_d1_d, D1, "b_d1_c")
        b_d2_c = load_col(b_d2_d, D2, "b_d2_c")
        b_d3_c = load_col(b_d3_d, OUT, "b_d3_c")

        # ---------------- x loads ----------------
        # xq[p, q, t, f] = x[128q+p, t, f]  (contiguous 2520B runs per (p,q))
        xq = singles.tile([128, QB, T, F], f32)
        nc.sync.dma_start(
            out=xq, in_=x_d[:, :, :].rearrange("(q p) t f -> p q t f", p=128))
        xqh = singles.tile([128, QB, T, F], f16)

        # ------- weights: load + PE-transpose; LN fold into layer-0 -------
        with tc.tile_pool(name="ps_pre", bufs=3, space="PSUM") as pre:
            def transpose_to(dst, src_ap, p, fdim):
                pt = pre.tile([fdim, p], f32, tag="scr", name="tr_ps")
                nc.tensor.transpose(pt, src_ap, ident[:p, :p])
                nc.vector.tensor_copy(out=dst, in_=pt)

            # stat constants, all-partition broadcast WITHOUT a DRAM
            # roundtrip: rhs columns hold per-h products; contracting with an
            # all-ones [128,128] stationary sums over h into every partition.
            NST = F * F + (F + 2) + (F + 1)
            rhs_all = small.tile([H, NST], f32, tag="rhs_all", name="rhs_all")
            for i in range(F):
                nc.vector.tensor_tensor(
                    out=rhs_all[:, i * F:(i + 1) * F], in0=w_in_raw,
                    in1=w_in_raw[:, i:i + 1].to_broadcast([H, F]), op=ALU.mult)
            nc.vector.tensor_scalar_mul(out=rhs_all[:, F * F:F * F + F],
                                        in0=w_in_raw, scalar1=b_in_c)
            nc.vector.tensor_tensor(out=rhs_all[:, F * F + F:F * F + F + 1],
                                    in0=b_in_c, in1=b_in_c, op=ALU.mult)
            nc.vector.tensor_scalar_mul(
                out=rhs_all[:, F * F + F + 1:F * F + F + 2], in0=b_in_c,
                scalar1=1.0)
            nc.vector.tensor_scalar_mul(
                out=rhs_all[:, F * F + F + 2:F * F + F + 2 + F], in0=w_in_raw,
                scalar1=1.0 / H)
            nc.vector.tensor_scalar_mul(out=rhs_all[:, NST - 1:NST],
                                        in0=b_in_c, scalar1=1.0 / H)
            ones128 = singles.tile([128, 128], f32)
            nc.vector.memset(ones128, 1.0)
            sbc_ps = pre.tile([128, NST], f32, tag="sbc", name="sbc_ps",
                              bufs=1)
            nc.tensor.matmul(sbc_ps, ones128, rhs_all, start=True, stop=True)
            # p_ws/wsn (partition-0 row) still needed for the LN weight fold
            p_ws = pre.tile([1, F + 1], f32, tag="scr", name="p_ws")
            nc.tensor.matmul(p_ws[:, 0:F], ones_col, w_in_raw, start=True,
                             stop=False, skip_group_check=True)
            nc.tensor.matmul(p_ws[:, F:F + 1], ones_col, b_in_c, start=False,
                             stop=True, skip_group_check=True)
            wsn = small.tile([1, F + 1], f32, tag="wsn", name="wsn")
            nc.vector.tensor_scalar_mul(out=wsn, in0=p_ws, scalar1=1.0 / H)
            sbc = singles.tile([128, NST], f32)
            nc.vector.tensor_copy(out=sbc, in_=sbc_ps)

            wihT0f = singles.tile([H, 4, H], f32)  # raw Wih0^T per gate
            wihT1 = singles.tile([H, 4, H], f16)
            whhT = [singles.tile([H, 4, H], f16, name=f"whhT{L}", tag=f"whhT{L}")
                    for L in range(2)]
            for L in range(2):
                for cc in range(4):
                    raw = trans.tile([H, H], f32, tag="u", name="raw")
                    nc.sync.dma_start(out=raw, in_=Wih_d[L][cc * H:(cc + 1) * H, :])
                    pt_w = pre.tile([H, H], f32, tag="scr", name="tr_ps_w")
                    nc.tensor.transpose(pt_w, raw, ident)
                    if L == 0:
                        nc.vector.tensor_copy(out=wihT0f[:, cc, :], in_=pt_w)
                    else:
                        nc.vector.tensor_copy(out=wihT1[:, cc, :], in_=pt_w)
                    raw2 = trans.tile([H, H], f32, tag="v_", name="raw2")
                    nc.sync.dma_start(out=raw2, in_=Whh_d[L][cc * H:(cc + 1) * H, :])
                    transpose_to(whhT[L][:, cc, :], raw2, H, H)

            # gate biases beff[L] [128, 4]; layer-0 gains Wih0 @ be_in
            beff = []
            for L in range(2):
                bt_ = singles.tile([H, 4], f32, name=f"beff{L}", tag=f"beff{L}")
                bih_sb = small.tile([H, 4], f32, tag="bload", name="bih_sb")
                nc.sync.dma_start(out=bih_sb,
                                  in_=bih_d[L][:].rearrange("(c p) -> p c", p=H))
                bhh_sb = small.tile([H, 4], f32, tag="bload2", name="bhh_sb")
                nc.sync.dma_start(out=bhh_sb,
                                  in_=bhh_d[L][:].rearrange("(c p) -> p c", p=H))
                nc.vector.tensor_add(out=bt_, in0=bih_sb, in1=bhh_sb)
                beff.append(bt_)
            for cc in range(4):
                pb = pre.tile([H, 1], f32, tag="scr", name="pb")
                nc.tensor.matmul(pb, wihT0f[:, cc, :], be_in_c, start=True,
                                 stop=True)
                nc.vector.tensor_add(out=beff[0][:, cc:cc + 1],
                                     in0=beff[0][:, cc:cc + 1], in1=pb)

            # ---- LN fold: Pg = diag(g_in) [W_in - 1 ws^T/H | b_in - bs/H] ----
            pw_bc = pre.tile([H, F + 1], f32, tag="scr", name="pw_bc")
            nc.tensor.matmul(pw_bc, ones_row[:, 0:H], wsn, start=True, stop=True)
            cat8 = small.tile([H, F + 1], f32, tag="cat8", name="cat8")
            nc.vector.tensor_copy(out=cat8[:, 0:F], in_=w_in_raw)
            nc.vector.tensor_copy(out=cat8[:, F:F + 1], in_=b_in_c)
            Pg = singles.tile([H, F + 1], f32)
            nc.vector.tensor_sub(out=Pg, in0=cat8, in1=pw_bc)
            nc.vector.tensor_scalar_mul(out=Pg, in0=Pg, scalar1=g_in_c)
            stat0 = singles.tile([F + 1, 4, H], f16)
            for cc in range(4):
                ps8 = pre.tile([F + 1, H], f32, tag="scr", name="ps8")
                nc.tensor.matmul(ps8, Pg, wihT0f[:, cc, :], start=True, stop=True)
                nc.vector.tensor_copy(out=stat0[:, cc, :], in_=ps8)

            # dense head weights (transposed, f16 stationaries)
            wd1T = singles.tile([H, D1], f16)
            wd1_raw = trans.tile([D1, H], f32, tag="u", name="wd1_raw")
            nc.sync.dma_start(out=wd1_raw, in_=W_d1_d[:, :])
            transpose_to(wd1T, wd1_raw, D1, H)
            wd2T = singles.tile([D1, D2], f16)
            wd2_raw = trans.tile([D2, D1], f32, tag="v_", name="wd2_raw")
            nc.sync.dma_start(out=wd2_raw, in_=W_d2_d[:, :])
            transpose_to(wd2T, wd2_raw, D2, D1)
            wd3T = singles.tile([D2, OUT], f16)
            wd3_raw = trans.tile([OUT, D2], f32, tag="u", name="wd3_raw")
            nc.sync.dma_start(out=wd3_raw, in_=W_d3_d[:, :])
            transpose_to(wd3T, wd3_raw, OUT, D2)

            # -------- prepass: LN stats in batch-major [128, (q t)] --------
            # per (t,b) sample:  sum_h p = ws.x + bs ;
            #   sum_h p^2 = x^T M x + 2 l^T x + c0  (M = W^T W, l = W^T b).
            # The stat scalars are per-sample CONSTANTS, so the stats run
            # directly on xq's own [128p, (q t)] layout -- no transposes --
            # and rstd lands directly in rT's batch-major layout.
            mbc = sbc[:, 0:F * F]
            lbc = sbc[:, F * F:F * F + F]
            c0bc = sbc[:, F * F + F:F * F + F + 1]
            wbc = sbc[:, F * F + F + 2:F * F + F + 2 + F]  # ws/H
            bshbc = sbc[:, NST - 1:NST]  # bs/H

            TS = T_steps

            def xf(fi):
                return xq[:, :, :TS, fi]

            rT = singles.tile([128, QB, T], f32)
            nmu = trans.tile([128, QB, T], f32, tag="sig_i", name="st_nmu")
            nc.vector.tensor_scalar_mul(out=nmu[:, :, :TS], in0=xf(0),
                                        scalar1=wbc[:, 0:1])
            for fi in range(1, F):
                nc.vector.scalar_tensor_tensor(
                    out=nmu[:, :, :TS], in0=xf(fi), scalar=wbc[:, fi:fi + 1],
                    in1=nmu[:, :, :TS], op0=ALU.mult, op1=ALU.add)
            # nmu = -(ws.x/H + bs/H)
            nc.vector.tensor_scalar(out=nmu[:, :, :TS], in0=nmu[:, :, :TS],
                                    scalar1=bshbc, scalar2=-1.0,
                                    op0=ALU.add, op1=ALU.mult)
            # quadratic form: y-chains (TensorScalarPtr, DVE-only) on DVE;
            # the x*y products and qacc accumulation (plain TensorTensor) on
            # Pool. Emission inline keeps the conservative cross-engine sem
            # joins tight.
            qacc = trans.tile([128, QB, T], f32, tag="x_pool_a", name="st_qacc")
            tprod = trans.tile([128, QB, T], f32, tag="x_pool_b",
                               name="st_tprod")
            for fi in range(F):
                yf = trans.tile([128, QB, T], f32, tag="st_yf", name="st_yf")
                nc.vector.tensor_scalar_mul(out=yf[:, :, :TS], in0=xf(0),
                                            scalar1=mbc[:, fi * F:fi * F + 1])
                for fj in range(1, F):
                    nc.vector.scalar_tensor_tensor(
                        out=yf[:, :, :TS], in0=xf(fj),
                        scalar=mbc[:, fi * F + fj:fi * F + fj + 1],
                        in1=yf[:, :, :TS], op0=ALU.mult, op1=ALU.add)
                if fi == 0:
                    nc.gpsimd.tensor_tensor(out=qacc[:, :, :TS], in0=xf(fi),
                                            in1=yf[:, :, :TS], op=ALU.mult)
                else:
                    nc.gpsimd.tensor_tensor(out=tprod[:, :, :TS], in0=xf(fi),
                                            in1=yf[:, :, :TS], op=ALU.mult)
                    nc.gpsimd.tensor_add(out=qacc[:, :, :TS],
                                         in0=qacc[:, :, :TS],
                                         in1=tprod[:, :, :TS])
            # + 2 l.x
            lin = trans.tile([128, QB, T], f32, tag="st_lin", name="st_lin")
            nc.vector.tensor_scalar_mul(out=lin[:, :, :TS], in0=xf(0),
                                        scalar1=lbc[:, 0:1])
            for fi in range(1, F):
                nc.vector.scalar_tensor_tensor(
                    out=lin[:, :, :TS], in0=xf(fi), scalar=lbc[:, fi:fi + 1],
                    in1=lin[:, :, :TS], op0=ALU.mult, op1=ALU.add)
            nc.vector.scalar_tensor_tensor(out=qacc[:, :, :TS],
                                           in0=lin[:, :, :TS],
                                           scalar=2.0, in1=qacc[:, :, :TS],
                                           op0=ALU.mult, op1=ALU.add)
            # var = (q + c0)/H - mu^2 ; rT = 1/sqrt(var+eps)
            nc.vector.tensor_scalar(out=qacc[:, :, :TS], in0=qacc[:, :, :TS],
                                    scalar1=c0bc, scalar2=1.0 / H,
                                    op0=ALU.add, op1=ALU.mult)
            musq = trans.tile([128, QB, T], f32, tag="st_yf", name="st_musq")
            nc.vector.tensor_tensor(out=musq[:, :, :TS], in0=nmu[:, :, :TS],
                                    in1=nmu[:, :, :TS], op=ALU.mult)
            nc.vector.tensor_sub(out=qacc[:, :, :TS], in0=qacc[:, :, :TS],
                                 in1=musq[:, :, :TS])
            nc.scalar.activation(out=rT[:, :, :TS], in_=qacc[:, :, :TS],
                                 func=AF.Sqrt, bias=eps_col, scale=1.0)
            nc.vector.reciprocal(out=rT[:, :, :TS], in_=rT[:, :, :TS])
            # f32 -> f16 x copy for the loop (Pool; runs during the DVE tail)
            nc.gpsimd.tensor_tensor(
                out=xqh[:, :, :, :].rearrange("p q t f -> p (q t f)"),
                in0=xq[:, :, :, :].rearrange("p q t f -> p (q t f)"),
                in1=ones_col[:, 0:1].to_broadcast([128, QB * T * F]),
                op=ALU.mult)

        # ---------------- states ----------------
        h1 = singles.tile([H, BL], f16, name="h1", tag="h1")
        c = [singles.tile([H, BL], cdt, name="c0", tag="c0"),
             singles.tile([H, BL], cdt, name="c1", tag="c1")]
        h0_z = trans.tile([H, BL], f16, tag="h0", name="h0_init", bufs=3)
        nc.vector.memset(h0_z, 0.0)
        nc.vector.memset(h1, 0.0)
        for L in range(2):
            nc.vector.memset(c[L], 0.0)
        # layer-1 runs TWO steps behind layer-0: every ACT op in a period then
        # depends only on >= half-period-old results, so the h0 recurrence
        # tail (tanh -> h-mult -> PE -> first gate ACT) hides under L1's ops.
        h0_hist = [None, h0_z]

        ps_pg = ctx.enter_context(tc.tile_pool(name="ps_pg", bufs=3, space="PSUM"))
        ps_px = ctx.enter_context(tc.tile_pool(name="ps_px", bufs=2, space="PSUM"))

        def pg_tile(shape, name):
            return ps_pg.tile(shape, f32, tag="pg", name=name)

        # ---------------- main loop ----------------
        def lstm_step(L, inp, hprev, hout, hh_first, split=False,
                      first_gate_split=False):
            sig_i = trans.tile([H, BL], f16, tag="sig_i", name="sig_i")
            sig_f = trans.tile([H, BL], f16, tag="sig_f", name="sig_f")
            tg = trans.tile([H, BL], f16, tag="tg", name="tg")
            sig_o = trans.tile([H, BL], f16, tag="sig_o", name="sig_o")
            outs = [sig_i, sig_f, tg, sig_o]
            funcs = [AF.Sigmoid, AF.Sigmoid, AF.Tanh, AF.Sigmoid]
            wih = stat0 if L == 0 else wihT1
            for gc in range(4):
                pg = pg_tile([H, BL], "pg_gates")
                for hc in range(NH):
                    sl = slice(hc * 512, (hc + 1) * 512)
                    ops = [(wih[:, gc, :], inp), (whhT[L][:, gc, :], hprev)]
                    if hh_first:
                        ops.reverse()
                    nc.tensor.matmul(pg[:, sl], ops[0][0], ops[0][1][:, sl],
                                     start=True, stop=False)
                    nc.tensor.matmul(pg[:, sl], ops[1][0], ops[1][1][:, sl],
                                     start=False, stop=True)
                if gc == 0 and first_gate_split:
                    for hc in range(NH):
                        sl = slice(hc * 512, (hc + 1) * 512)
                        nc.scalar.activation(out=outs[gc][:, sl],
                                             in_=pg[:, sl], func=funcs[gc],
                                             bias=beff[L][:, gc:gc + 1],
                                             scale=1.0)
                else:
                    nc.scalar.activation(out=outs[gc], in_=pg, func=funcs[gc],
                                         bias=beff[L][:, gc:gc + 1], scale=1.0)
            u = trans.tile([H, BL], f16, tag="u", name="u")
            v_ = trans.tile([H, BL], cdt, tag="v_", name="v_")
            tc_ = trans.tile([H, BL], f16, tag="tc_", name="tc_")
            if not split:
                nc.vector.tensor_tensor(out=v_, in0=sig_f, in1=c[L], op=ALU.mult)
                nc.vector.tensor_tensor(out=u, in0=sig_i, in1=tg, op=ALU.mult)
                nc.vector.tensor_add(out=c[L], in0=u, in1=v_)
                nc.scalar.activation(out=tc_, in_=c[L], func=AF.Tanh, scale=1.0)
                nc.vector.tensor_tensor(out=hout, in0=sig_o, in1=tc_, op=ALU.mult)
            else:
                # half-column tail: lets tanh/h pipeline against the DVE chain
                # v-products first: they only need sig_f (2nd ACT op)
                for hc in range(NH):
                    sl = slice(hc * 512, (hc + 1) * 512)
                    nc.vector.tensor_tensor(out=v_[:, sl], in0=sig_f[:, sl],
                                            in1=c[L][:, sl], op=ALU.mult)
                for hc in range(NH):
                    sl = slice(hc * 512, (hc + 1) * 512)
                    nc.vector.tensor_tensor(out=u[:, sl], in0=sig_i[:, sl],
                                            in1=tg[:, sl], op=ALU.mult)
                    nc.vector.tensor_add(out=c[L][:, sl], in0=u[:, sl],
                                         in1=v_[:, sl])
                for hc in range(NH):
                    sl = slice(hc * 512, (hc + 1) * 512)
                    nc.scalar.activation(out=tc_[:, sl], in_=c[L][:, sl],
                                         func=AF.Tanh, scale=1.0)
                for hc in range(NH):
                    sl = slice(hc * 512, (hc + 1) * 512)
                    nc.vector.tensor_tensor(out=hout[:, sl], in0=sig_o[:, sl],
                                            in1=tc_[:, sl], op=ALU.mult)

        for t in range(T_steps):
            # scale x_t by rstd in batch-major layout (Pool), lane 7 = rstd
            xs = xsp.tile([128, QB, F + 1], f16, tag="xs", name="xs")
            nc.gpsimd.tensor_tensor(
                out=xs[:, :, 0:F], in0=xqh[:, :, t, :],
                in1=rT[:, :, t:t + 1].to_broadcast([128, QB, F]), op=ALU.mult)
            nc.gpsimd.tensor_tensor(out=xs[:, :, F], in0=rT[:, :, t],
                                    in1=ones_q, op=ALU.mult)
            # PE transpose to [8, BL] fp16 moving operand xt = [x r ; r]
            xt = trans.tile([F + 1, BL], f16, tag="xt", name="xt")
            for half in range(2):
                px = ps_px.tile([F + 1, 512], f16, tag="pxt", name="pxt")
                for qi in range(4):
                    q = half * 4 + qi
                    nc.tensor.transpose(px[:, qi * 128:(qi + 1) * 128],
                                        xs[:, q, :], ident16)
                nc.vector.tensor_copy(
                    out=xt[:, half * 512:(half + 1) * 512], in_=px)
            if t > 1:
                lstm_step(1, h0_hist[0], h1, h1, hh_first=True)
            h0_new = trans.tile([H, BL], f16, tag="h0", name="h0_new", bufs=3)
            lstm_step(0, xt, h0_hist[1], h0_new, hh_first=False, split=False)
            h0_hist = [h0_hist[1], h0_new]
            if dbg and t == 0:
                xtc = trans.tile([F + 1, BL], f32, tag="v_", name="xtc_dbg")
                nc.vector.tensor_copy(out=xtc, in_=xt)
                nc.sync.dma_start(out=dbg_xt[:, :], in_=xtc)
                h0c = trans.tile([H, BL], f32, tag="u", name="h0c_dbg")
                nc.vector.tensor_copy(out=h0c, in_=h0_new)
                nc.sync.dma_start(out=dbg_h0[:, :], in_=h0c)
                c0c = trans.tile([H, BL], f32, tag="tc_", name="c0c_dbg")
                nc.vector.tensor_copy(out=c0c, in_=c[0])
                nc.sync.dma_start(out=dbg_c0[:, :], in_=c0c)
        lstm_step(1, h0_hist[0], h1, h1, hh_first=True, split=True,
                  first_gate_split=True)
        lstm_step(1, h0_hist[1], h1, h1, hh_first=True, split=True,
                  first_gate_split=True)

        # ---------------- head ----------------
        sqh = trans.tile([H, BL], f16, tag="sig_f", name="sqh")
        nc.vector.tensor_tensor(out=sqh, in0=h1, in1=h1, op=ALU.mult)
        ps_s1 = pg_tile([1, BL], "ps_s1")
        ps_s2 = pg_tile([1, BL], "ps_s2")
        for hc in range(NH):
            sl = slice(hc * 512, (hc + 1) * 512)
            nc.tensor.matmul(ps_s1[:, sl], ones_col16, h1[:, sl],
                             start=True, stop=True, skip_group_check=True)
            nc.tensor.matmul(ps_s2[:, sl], ones_col16, sqh[:, sl],
                             start=True, stop=True, skip_group_check=True)
        # head LN stats: scale ops on ACT (f16 out), multiplies on DVE at 2x
        nm16 = singles.tile([1, BL], f16, tag="nm16", name="nm16")
        nc.scalar.activation(out=nm16, in_=ps_s1, func=AF.Copy,
                             scale=-1.0 / H)
        v16 = singles.tile([1, BL], f16, tag="v16", name="v16")
        nc.scalar.activation(out=v16, in_=ps_s2, func=AF.Copy, scale=1.0 / H)
        musq_h = singles.tile([1, BL], f16, tag="musq", name="musq_h")
        nc.vector.tensor_tensor(out=musq_h, in0=nm16, in1=nm16, op=ALU.mult)
        with nc.allow_low_precision(reason="head LN var in f16; |var|~O(1)"):
            nc.vector.tensor_sub(out=v16, in0=v16, in1=musq_h)
        rh16 = singles.tile([1, BL], f16, tag="rh16", name="rh16")
        nc.scalar.activation(out=rh16, in_=v16, func=AF.Sqrt,
                             bias=eps_col[0:1], scale=1.0)
        with nc.allow_low_precision(reason="head LN rstd in f16"):
            nc.vector.reciprocal(out=rh16, in_=rh16)
        pnm = pg_tile([H, BL], "pnm")
        prh = ps_px.tile([H, 512], f32, tag="pxt", name="prh0")
        prh2 = ps_px.tile([H, 512], f32, tag="pxt", name="prh1")
        prhs = [prh, prh2]
        for hc in range(NH):
            sl = slice(hc * 512, (hc + 1) * 512)
            nc.tensor.matmul(pnm[:, sl], ones_row128_16, nm16[:, sl],
                             start=True, stop=True, skip_group_check=True)
            nc.tensor.matmul(prhs[hc], ones_row128_16, rh16[:, sl],
                             start=True, stop=True, skip_group_check=True)
        t1 = trans.tile([H, BL], f32, tag="tg", name="t1")
        t2 = trans.tile([H, BL], f32, tag="sig_o", name="t2")
        last = trans.tile([H, BL], f16, tag="u", name="last")
        for hc in range(NH):
            sl = slice(hc * 512, (hc + 1) * 512)
            nc.vector.tensor_tensor(out=t1[:, sl], in0=h1[:, sl],
                                    in1=pnm[:, sl], op=ALU.add)
            nc.vector.tensor_tensor(out=t2[:, sl], in0=t1[:, sl], in1=prhs[hc],
                                    op=ALU.mult)
            nc.vector.tensor_scalar(out=last[:, sl], in0=t2[:, sl],
                                    scalar1=g_ln_c, scalar2=be_ln_c,
                                    op0=ALU.mult, op1=ALU.add)
        pd1 = pg_tile([D1, BL], "pd1")
        for hc in range(NH):
            sl = slice(hc * 512, (hc + 1) * 512)
            nc.tensor.matmul(pd1[:, sl], wd1T, last[:, sl], start=True, stop=True,
                             skip_group_check=True)
        d1 = trans.tile([D1, BL], f16, tag="v_", name="d1")
        for hc in range(NH):
            sl = slice(hc * 512, (hc + 1) * 512)
            nc.scalar.activation(out=d1[:, sl], in_=pd1[:, sl],
                                 func=AF.Relu, bias=b_d1_c, scale=1.0)
        pd2 = pg_tile([D2, BL], "pd2")
        for hc in range(NH):
            sl = slice(hc * 512, (hc + 1) * 512)
            nc.tensor.matmul(pd2[:, sl], wd2T, d1[:, sl], start=True, stop=True,
                             skip_group_check=True)
        d2 = trans.tile([D2, BL], f16, tag="tc_", name="d2")
        for hc in range(NH):
            sl = slice(hc * 512, (hc + 1) * 512)
            nc.scalar.activation(out=d2[:, sl], in_=pd2[:, sl],
                                 func=AF.Relu, bias=b_d2_c, scale=1.0)
        pd3 = pg_tile([OUT, BL], "pd3")
        for hc in range(NH):
            sl = slice(hc * 512, (hc + 1) * 512)
            nc.tensor.matmul(pd3[:, sl], wd3T, d2[:, sl], start=True, stop=True,
                             skip_group_check=True)
        o3 = trans.tile([OUT, BL], f32, tag="sig_f", name="o3")
        for hc in range(NH):
            sl = slice(hc * 512, (hc + 1) * 512)
            nc.scalar.activation(out=o3[:, sl], in_=pd3[:, sl],
                                 func=AF.Identity, bias=b_d3_c, scale=1.0)
        outT = singles.tile([128, QB, OUT], f32)
        for q in range(QB):
            pot = ps_px.tile([128, OUT], f32, tag="pxt", name="pot")
            nc.tensor.transpose(pot, o3[:, q * 128:(q + 1) * 128],
                                ident[:OUT, :OUT])
            nc.vector.tensor_copy(out=outT[:, q, :], in_=pot)
        nc.sync.dma_start(
            out=out_d[:, :].rearrange("(q p) c -> p q c", p=128),
            in_=outT)
    return nc


_CACHE = {}


def _get_runner(T_steps=T):
    if "runner" in _CACHE:
        return _CACHE["runner"]
    import jax
    from jax.sharding import Mesh, PartitionSpec
    from jax.experimental.shard_map import shard_map
    import concourse.bacc as bacc
    import concourse.mybir as mybir
    from concourse.bass2jax import install_neuronx_cc_hook, _bass_exec_p, \
        partition_id_tensor

    nc = bacc.Bacc()
    _build(nc, T_steps=T_steps)
    nc.compile()
    install_neuronx_cc_hook()

    partition_name = nc.partition_id_tensor.name if nc.partition_id_tensor else None
    in_names, out_names, out_avals, zero_outs = [], [], [], []
    for alloc in nc.m.functions[0].allocations:
        if not isinstance(alloc, mybir.MemoryLocationSet):
            continue
        name = alloc.memorylocations[0].name
        if alloc.kind == "ExternalInput":
            if name != partition_name:
                in_names.append(name)
        elif alloc.kind == "ExternalOutput":
            out_names.append(name)
            shape = tuple(alloc.tensor_shape)
            dtype = mybir.dt.np(alloc.dtype)
            out_avals.append(jax.core.ShapedArray(shape, dtype))
            zero_outs.append(np.zeros(shape, dtype))
    n_params = len(in_names)
    all_in_names = in_names + out_names + ([partition_name] if partition_name else [])

    def _body(*args):
        operands = list(args)
        if partition_name is not None:
            operands.append(partition_id_tensor())
        outs = _bass_exec_p.bind(
            *operands,
            out_avals=tuple(out_avals),
            in_names=tuple(all_in_names),
            out_names=tuple(out_names),
            lowering_input_output_aliases=(),
            sim_require_finite=False,
            sim_require_nnan=False,
            nc=nc,
        )
        return tuple(outs)

    devices = jax.devices()[:NCORES]
    mesh = Mesh(np.asarray(devices), ("core",))
    in_specs = (PartitionSpec("core"),) * (n_params + len(out_names))
    out_specs = (PartitionSpec("core"),) * len(out_names)
    sharded = jax.jit(
        shard_map(_body, mesh=mesh, in_specs=in_specs, out_specs=out_specs,
                  check_rep=False),
        keep_unused=True)
    _CACHE["runner"] = (sharded, in_names, out_names, zero_outs)
    return _CACHE["runner"]


def kernel(**inputs) -> np.ndarray:
    sharded, in_names, out_names, zero_outs = _get_runner()
    inp = {k: np.ascontiguousarray(np.asarray(v), dtype=np.float32)
           for k, v in inputs.items()}

    def core_val(name, ci):
        if name == "x":
            return inp["x"][ci * BL:(ci + 1) * BL]
        return inp[name]

    concat_in = [
        np.concatenate([core_val(n, ci) for ci in range(NCORES)], axis=0)
        for n in in_names
    ]
    concat_zeros = [
        np.zeros((NCORES * z.shape[0], *z.shape[1:]), z.dtype) for z in zero_outs
    ]
    import jax
    out_arrs = sharded(*concat_in, *concat_zeros)
    jax.block_until_ready(out_arrs)
    oi = out_names.index("out")
    full = np.asarray(out_arrs[oi]).reshape(B, OUT)
    return full.astype(np.float32)


# revision 40
# speedup vs baseline: 1.0625x; 1.0071x over previous
"""DepletionLSTM Trainium2 kernel (v2).

Self-contained: builds a Bass/Tile kernel for the 2-layer-LSTM network,
shards the batch over 8 NeuronCores (pure data parallelism), runs via
PJRT/axon, returns the full [8192, 30] float32 output.

Strategy (per core, 1024 batch):
- All activations SBUF-resident; zero in-loop DRAM traffic.
- The input-projection LayerNorm is folded INTO the layer-0 gate weights:
  x0 = (W_in x + b_in - mu 1) r  ==  [W'|b'] @ [x r ; r]  with
  W' = W_in - 1 ws^T/H, b' = b_in - bs/H, so the layer-0 input-gate matmul
  uses an 8-row stationary Stat0 = ([W'|b'])^T diag(g_in) Wih0^T and the
  8-row moving operand xt = [x r ; r].  No separate projection matmul, no
  x0 tile, no PSUM->SBUF projection copy.
- rstd (r) is applied in batch-major layout BEFORE the PE transpose: a Pool
  (gpsimd) op scales x_t[128p, 8q, 7f] by rT[:, :, t] (0-stride broadcast
  over f) and writes r itself into lane 7, then 8 PE transposes produce the
  [8, BL] fp16 moving operand.  All per-step DRAM broadcast DMAs are gone.
- fp16 everywhere on matmul operands and the elementwise chain: DVE runs in
  2x mode (594ns per [128,1024] op vs 1127 fp32); cell state c stays fp32.
- LN stats prepass runs directly in xq's batch-major [128, (q t)] layout
  via the quadratic-form identity (the stat scalars are per-sample
  constants, so no transposes are needed and rstd lands directly in rT's
  layout); stat constants are summed over h with an all-ones stationary
  matmul (no DRAM staging). y-chains on DVE, products/accumulation on Pool,
  with emission interleaved because cross-engine tile deps degrade to
  engine-counter joins.
- Layer 1 runs TWO timesteps behind layer 0: every ACT op in a steady-state
  period then depends only on >=half-period-old results, so the h0
  recurrence tail (tanh -> h-mult -> PE -> first gate ACT) hides entirely
  under L1's gate ops -- the ACT engine runs gap-free at its 10x1038ns/step
  floor.  All gate activations live in one ACT table (no table loads).

PSUM: "pg" gates 3x[128,1024] (6 banks), "px" x-transposes 2x[8,512]
(2 banks); prepass uses a separate pool that closes before the loop.
"""
import sys
sys.path.insert(0, '/opt/trn_rl_repo')

import numpy as np

B, T, F, H, D1, D2, OUT = 8192, 90, 7, 128, 128, 64, 30
NCORES = 8
BL = B // NCORES
G4 = 4 * H
NH = BL // 512
QB = BL // 128
EPS = 1e-5
C_F16 = True


def _build(nc, T_steps=T, dbg=False):
    import concourse.tile as tile
    from concourse import mybir
    from concourse.masks import make_identity

    f32 = mybir.dt.float32
    f16 = mybir.dt.float16
    AF = mybir.ActivationFunctionType
    ALU = mybir.AluOpType
    cdt = f16 if C_F16 else f32

    # ---------------- DRAM I/O ----------------
    x_d = nc.dram_tensor("x", [BL, T, F], f32, kind="ExternalInput")
    W_in_d = nc.dram_tensor("W_in", [H, F], f32, kind="ExternalInput")
    b_in_d = nc.dram_tensor("b_in", [H], f32, kind="ExternalInput")
    g_in_d = nc.dram_tensor("g_in", [H], f32, kind="ExternalInput")
    be_in_d = nc.dram_tensor("be_in", [H], f32, kind="ExternalInput")
    Wih_d = [nc.dram_tensor("Wih0", [G4, H], f32, kind="ExternalInput"),
             nc.dram_tensor("Wih1", [G4, H], f32, kind="ExternalInput")]
    Whh_d = [nc.dram_tensor("Whh0", [G4, H], f32, kind="ExternalInput"),
             nc.dram_tensor("Whh1", [G4, H], f32, kind="ExternalInput")]
    bih_d = [nc.dram_tensor("bih0", [G4], f32, kind="ExternalInput"),
             nc.dram_tensor("bih1", [G4], f32, kind="ExternalInput")]
    bhh_d = [nc.dram_tensor("bhh0", [G4], f32, kind="ExternalInput"),
             nc.dram_tensor("bhh1", [G4], f32, kind="ExternalInput")]
    g_ln_d = nc.dram_tensor("g_ln", [H], f32, kind="ExternalInput")
    be_ln_d = nc.dram_tensor("be_ln", [H], f32, kind="ExternalInput")
    W_d1_d = nc.dram_tensor("W_d1", [D1, H], f32, kind="ExternalInput")
    b_d1_d = nc.dram_tensor("b_d1", [D1], f32, kind="ExternalInput")
    W_d2_d = nc.dram_tensor("W_d2", [D2, D1], f32, kind="ExternalInput")
    b_d2_d = nc.dram_tensor("b_d2", [D2], f32, kind="ExternalInput")
    W_d3_d = nc.dram_tensor("W_d3", [OUT, D2], f32, kind="ExternalInput")
    b_d3_d = nc.dram_tensor("b_d3", [OUT], f32, kind="ExternalInput")
    out_d = nc.dram_tensor("out", [BL, OUT], f32, kind="ExternalOutput")
    if dbg:
        dbg_xt = nc.dram_tensor("dbg_xt", [8, BL], f32, kind="ExternalOutput")
        dbg_h0 = nc.dram_tensor("dbg_h0", [H, BL], f32, kind="ExternalOutput")
        dbg_c0 = nc.dram_tensor("dbg_c0", [H, BL], f32, kind="ExternalOutput")
        dbg_r = nc.dram_tensor("dbg_r", [T, BL], f32, kind="ExternalOutput")

    import contextlib
    with tile.TileContext(nc) as tc, contextlib.ExitStack() as ctx:
        singles = ctx.enter_context(tc.tile_pool(name="singles", bufs=1))
        trans = ctx.enter_context(tc.tile_pool(name="trans", bufs=2))
        small = ctx.enter_context(tc.tile_pool(name="small", bufs=2))
        xsp = ctx.enter_context(tc.tile_pool(name="xsp", bufs=3))
        dpool = ctx.enter_context(tc.tile_pool(name="dpool", bufs=1, space="DRAM"))

        # ---------------- constants ----------------
        ident = singles.tile([128, 128], f32)
        make_identity(nc, ident)
        ident16 = singles.tile([128, 128], f16)
        make_identity(nc, ident16)
        ones_row = singles.tile([1, 512], f32)
        nc.vector.memset(ones_row, 1.0)
        ones_col = singles.tile([128, 1], f32)
        nc.vector.memset(ones_col, 1.0)
        ones_col16 = singles.tile([128, 1], f16)
        nc.vector.memset(ones_col16, 1.0)
        ones_row90 = singles.tile([1, T], f32)
        nc.vector.memset(ones_row90, 1.0)
        ones_row128_16 = singles.tile([1, 128], f16)
        nc.vector.memset(ones_row128_16, 1.0)
        eps_col = singles.tile([128, 1], f32)
        nc.vector.memset(eps_col, EPS)
        ones_q = singles.tile([128, QB], f32)
        nc.vector.memset(ones_q, 1.0)

        def load_col(dram_vec, n, name):
            t_ = singles.tile([n, 1], f32, name=name, tag=name)
            nc.sync.dma_start(out=t_, in_=dram_vec[:].rearrange("(p o) -> p o", o=1))
            return t_

        w_in_raw = singles.tile([H, F], f32)
        nc.sync.dma_start(out=w_in_raw, in_=W_in_d[:, :])
        b_in_c = load_col(b_in_d, H, "b_in_c")
        g_in_c = load_col(g_in_d, H, "g_in_c")
        be_in_c = load_col(be_in_d, H, "be_in_c")
        g_ln_c = load_col(g_ln_d, H, "g_ln_c")
        be_ln_c = load_col(be_ln_d, H, "be_ln_c")
        b_d1_c = load_col(b_d1_d, D1, "b_d1_c")
        b_d2_c = load_col(b_d2_d, D2, "b_d2_c")
        b_d3_c = load_col(b_d3_d, OUT, "b_d3_c")

        # ---------------- x loads ----------------
        # xq[p, q, t, f] = x[128q+p, t, f]  (contiguous 2520B runs per (p,q))
        xq = singles.tile([128, QB, T, F], f32)
        nc.sync.dma_start(
            out=xq, in_=x_d[:, :, :].rearrange("(q p) t f -> p q t f", p=128))
        xqh = singles.tile([128, QB, T, F], f16)

        # ------- weights: load + PE-transpose; LN fold into layer-0 -------
        with tc.tile_pool(name="ps_pre", bufs=3, space="PSUM") as pre:
            def transpose_to(dst, src_ap, p, fdim):
                pt = pre.tile([fdim, p], f32, tag="scr", name="tr_ps")
                nc.tensor.transpose(pt, src_ap, ident[:p, :p])
                nc.vector.tensor_copy(out=dst, in_=pt)

            # stat constants, all-partition broadcast WITHOUT a DRAM
            # roundtrip: rhs columns hold per-h products; contracting with an
            # all-ones [128,128] stationary sums over h into every partition.
            NST = F * F + (F + 2) + (F + 1) + F + 1  # + M_ii/2 cols + c0/2
            rhs_all = small.tile([H, NST], f32, tag="rhs_all", name="rhs_all")
            NB = F * F + (F + 2) + (F + 1)
            for i in range(F):
                nc.vector.tensor_tensor(
                    out=rhs_all[:, i * F:(i + 1) * F], in0=w_in_raw,
                    in1=w_in_raw[:, i:i + 1].to_broadcast([H, F]), op=ALU.mult)
            nc.vector.tensor_scalar_mul(out=rhs_all[:, F * F:F * F + F],
                                        in0=w_in_raw, scalar1=b_in_c)
            nc.vector.tensor_tensor(out=rhs_all[:, F * F + F:F * F + F + 1],
                                    in0=b_in_c, in1=b_in_c, op=ALU.mult)
            nc.vector.tensor_scalar_mul(
                out=rhs_all[:, F * F + F + 1:F * F + F + 2], in0=b_in_c,
                scalar1=1.0)
            nc.vector.tensor_scalar_mul(
                out=rhs_all[:, F * F + F + 2:F * F + F + 2 + F], in0=w_in_raw,
                scalar1=1.0 / H)
            nc.vector.tensor_scalar_mul(out=rhs_all[:, NB - 1:NB],
                                        in0=b_in_c, scalar1=1.0 / H)
            nc.vector.tensor_tensor(out=rhs_all[:, NB:NB + F], in0=w_in_raw,
                                    in1=w_in_raw, op=ALU.mult)
            nc.vector.tensor_scalar_mul(out=rhs_all[:, NB:NB + F],
                                        in0=rhs_all[:, NB:NB + F], scalar1=0.5)
            nc.vector.tensor_scalar_mul(out=rhs_all[:, NB + F:NST],
                                        in0=rhs_all[:, F * F + F:F * F + F + 1],
                                        scalar1=0.5)
            ones128 = singles.tile([128, 128], f32)
            nc.vector.memset(ones128, 1.0)
            sbc_ps = pre.tile([128, NST], f32, tag="sbc", name="sbc_ps",
                              bufs=1)
            nc.tensor.matmul(sbc_ps, ones128, rhs_all, start=True, stop=True)
            # p_ws/wsn (partition-0 row) still needed for the LN weight fold
            p_ws = pre.tile([1, F + 1], f32, tag="scr", name="p_ws")
            nc.tensor.matmul(p_ws[:, 0:F], ones_col, w_in_raw, start=True,
                             stop=False, skip_group_check=True)
            nc.tensor.matmul(p_ws[:, F:F + 1], ones_col, b_in_c, start=False,
                             stop=True, skip_group_check=True)
            wsn = small.tile([1, F + 1], f32, tag="wsn", name="wsn")
            nc.vector.tensor_scalar_mul(out=wsn, in0=p_ws, scalar1=1.0 / H)
            sbc = singles.tile([128, NST], f32)
            nc.vector.tensor_copy(out=sbc, in_=sbc_ps)

            wihT0f = singles.tile([H, 4, H], f32)  # raw Wih0^T per gate
            wihT1 = singles.tile([H, 4, H], f16)
            whhT = [singles.tile([H, 4, H], f16, name=f"whhT{L}", tag=f"whhT{L}")
                    for L in range(2)]
            for L in range(2):
                for cc in range(4):
                    raw = trans.tile([H, H], f32, tag="u", name="raw")
                    nc.sync.dma_start(out=raw, in_=Wih_d[L][cc * H:(cc + 1) * H, :])
                    pt_w = pre.tile([H, H], f32, tag="scr", name="tr_ps_w")
                    nc.tensor.transpose(pt_w, raw, ident)
                    if L == 0:
                        nc.vector.tensor_copy(out=wihT0f[:, cc, :], in_=pt_w)
                    else:
                        nc.vector.tensor_copy(out=wihT1[:, cc, :], in_=pt_w)
                    raw2 = trans.tile([H, H], f32, tag="v_", name="raw2")
                    nc.sync.dma_start(out=raw2, in_=Whh_d[L][cc * H:(cc + 1) * H, :])
                    transpose_to(whhT[L][:, cc, :], raw2, H, H)

            # gate biases beff[L] [128, 4]; layer-0 gains Wih0 @ be_in
            beff = []
            for L in range(2):
                bt_ = singles.tile([H, 4], f32, name=f"beff{L}", tag=f"beff{L}")
                bih_sb = small.tile([H, 4], f32, tag="bload", name="bih_sb")
                nc.sync.dma_start(out=bih_sb,
                                  in_=bih_d[L][:].rearrange("(c p) -> p c", p=H))
                bhh_sb = small.tile([H, 4], f32, tag="bload2", name="bhh_sb")
                nc.sync.dma_start(out=bhh_sb,
                                  in_=bhh_d[L][:].rearrange("(c p) -> p c", p=H))
                nc.vector.tensor_add(out=bt_, in0=bih_sb, in1=bhh_sb)
                beff.append(bt_)
            for cc in range(4):
                pb = pre.tile([H, 1], f32, tag="scr", name="pb")
                nc.tensor.matmul(pb, wihT0f[:, cc, :], be_in_c, start=True,
                                 stop=True)
                nc.vector.tensor_add(out=beff[0][:, cc:cc + 1],
                                     in0=beff[0][:, cc:cc + 1], in1=pb)

            # ---- LN fold: Pg = diag(g_in) [W_in - 1 ws^T/H | b_in - bs/H] ----
            pw_bc = pre.tile([H, F + 1], f32, tag="scr", name="pw_bc")
            nc.tensor.matmul(pw_bc, ones_row[:, 0:H], wsn, start=True, stop=True)
            cat8 = small.tile([H, F + 1], f32, tag="cat8", name="cat8")
            nc.vector.tensor_copy(out=cat8[:, 0:F], in_=w_in_raw)
            nc.vector.tensor_copy(out=cat8[:, F:F + 1], in_=b_in_c)
            Pg = singles.tile([H, F + 1], f32)
            nc.vector.tensor_sub(out=Pg, in0=cat8, in1=pw_bc)
            nc.vector.tensor_scalar_mul(out=Pg, in0=Pg, scalar1=g_in_c)
            stat0 = singles.tile([F + 1, 4, H], f16)
            for cc in range(4):
                ps8 = pre.tile([F + 1, H], f32, tag="scr", name="ps8")
                nc.tensor.matmul(ps8, Pg, wihT0f[:, cc, :], start=True, stop=True)
                nc.vector.tensor_copy(out=stat0[:, cc, :], in_=ps8)

            # dense head weights (transposed, f16 stationaries)
            wd1T = singles.tile([H, D1], f16)
            wd1_raw = trans.tile([D1, H], f32, tag="u", name="wd1_raw")
            nc.sync.dma_start(out=wd1_raw, in_=W_d1_d[:, :])
            transpose_to(wd1T, wd1_raw, D1, H)
            wd2T = singles.tile([D1, D2], f16)
            wd2_raw = trans.tile([D2, D1], f32, tag="v_", name="wd2_raw")
            nc.sync.dma_start(out=wd2_raw, in_=W_d2_d[:, :])
            transpose_to(wd2T, wd2_raw, D2, D1)
            wd3T = singles.tile([D2, OUT], f16)
            wd3_raw = trans.tile([OUT, D2], f32, tag="u", name="wd3_raw")
            nc.sync.dma_start(out=wd3_raw, in_=W_d3_d[:, :])
            transpose_to(wd3T, wd3_raw, OUT, D2)

            # -------- prepass: LN stats in batch-major [128, (q t)] --------
            # per (t,b) sample:  sum_h p = ws.x + bs ;
            #   sum_h p^2 = x^T M x + 2 l^T x + c0  (M = W^T W, l = W^T b).
            # The stat scalars are per-sample CONSTANTS, so the stats run
            # directly on xq's own [128p, (q t)] layout -- no transposes --
            # and rstd lands directly in rT's batch-major layout.
            mbc = sbc[:, 0:F * F]
            lbc = sbc[:, F * F:F * F + F]
            c0bc = sbc[:, F * F + F:F * F + F + 1]
            wbc = sbc[:, F * F + F + 2:F * F + F + 2 + F]  # ws/H
            bshbc = sbc[:, NB - 1:NB]  # bs/H
            mhalf = sbc[:, NB:NB + F]  # M_ii/2
            c0half = sbc[:, NB + F:NST]  # |b|^2 / 2

            TS = T_steps

            def xf(fi):
                return xq[:, :, :TS, fi]

            rT = singles.tile([128, QB, T], f32)
            nmu = trans.tile([128, QB, T], f32, tag="sig_i", name="st_nmu")
            nc.vector.tensor_scalar_mul(out=nmu[:, :, :TS], in0=xf(0),
                                        scalar1=wbc[:, 0:1])
            for fi in range(1, F):
                nc.vector.scalar_tensor_tensor(
                    out=nmu[:, :, :TS], in0=xf(fi), scalar=wbc[:, fi:fi + 1],
                    in1=nmu[:, :, :TS], op0=ALU.mult, op1=ALU.add)
            # nmu = -(ws.x/H + bs/H)
            nc.vector.tensor_scalar(out=nmu[:, :, :TS], in0=nmu[:, :, :TS],
                                    scalar1=bshbc, scalar2=-1.0,
                                    op0=ALU.add, op1=ALU.mult)
            # l.x chain (DVE, runs during Pool's product burst)
            lin = trans.tile([128, QB, T], f32, tag="st_lin", name="st_lin")
            nc.vector.tensor_scalar_mul(out=lin[:, :, :TS], in0=xf(0),
                                        scalar1=lbc[:, 0:1])
            for fi in range(1, F):
                nc.vector.scalar_tensor_tensor(
                    out=lin[:, :, :TS], in0=xf(fi), scalar=lbc[:, fi:fi + 1],
                    in1=lin[:, :, :TS], op0=ALU.mult, op1=ALU.add)
            # quadratic form via symmetric expansion:
            #   x^T M x = 2*(sum_{i<j} M_ij x_i x_j + sum_i (M_ii/2) x_i^2)
            # The 28 products depend only on x, so Pool computes them all
            # while DVE runs the acc/lin chains; DVE then accumulates with
            # 28 TensorScalarPtr ops (vs 49 for the y-chain form).
            qacc = trans.tile([128, QB, T], f32, tag="x_pool_a", name="st_qacc")
            pairs = [(i, i) for i in range(F)] + \
                    [(i, j) for i in range(F) for j in range(i + 1, F)]
            first = True
            for (i, j) in pairs:
                pk = trans.tile([128, QB, T], f32, tag="prod", name="pk",
                                bufs=8)
                nc.gpsimd.tensor_tensor(out=pk[:, :, :TS], in0=xf(i),
                                        in1=xf(j), op=ALU.mult)
                scal = mhalf[:, i:i + 1] if i == j \
                    else mbc[:, i * F + j:i * F + j + 1]
                if first:
                    nc.vector.tensor_scalar_mul(out=qacc[:, :, :TS],
                                                in0=pk[:, :, :TS],
                                                scalar1=scal)
                    first = False
                else:
                    nc.vector.scalar_tensor_tensor(
                        out=qacc[:, :, :TS], in0=pk[:, :, :TS], scalar=scal,
                        in1=qacc[:, :, :TS], op0=ALU.mult, op1=ALU.add)
            nc.vector.tensor_add(out=qacc[:, :, :TS], in0=lin[:, :, :TS],
                                 in1=qacc[:, :, :TS])
            # var = 2*(q' + l.x + c0/2)/H - mu^2 ; rT = 1/sqrt(var+eps)
            nc.vector.tensor_scalar(out=qacc[:, :, :TS], in0=qacc[:, :, :TS],
                                    scalar1=c0half, scalar2=2.0 / H,
                                    op0=ALU.add, op1=ALU.mult)
            musq = trans.tile([128, QB, T], f32, tag="st_yf", name="st_musq")
            nc.vector.tensor_tensor(out=musq[:, :, :TS], in0=nmu[:, :, :TS],
                                    in1=nmu[:, :, :TS], op=ALU.mult)
            nc.vector.tensor_sub(out=qacc[:, :, :TS], in0=qacc[:, :, :TS],
                                 in1=musq[:, :, :TS])
            nc.scalar.activation(out=rT[:, :, :TS], in_=qacc[:, :, :TS],
                                 func=AF.Sqrt, bias=eps_col, scale=1.0)
            nc.vector.reciprocal(out=rT[:, :, :TS], in_=rT[:, :, :TS])
            # f32 -> f16 x copy for the loop (Pool; runs during the DVE tail)
            nc.gpsimd.tensor_tensor(
                out=xqh[:, :, :, :].rearrange("p q t f -> p (q t f)"),
                in0=xq[:, :, :, :].rearrange("p q t f -> p (q t f)"),
                in1=ones_col[:, 0:1].to_broadcast([128, QB * T * F]),
                op=ALU.mult)

        # ---------------- states ----------------
        h1 = singles.tile([H, BL], f16, name="h1", tag="h1")
        c = [singles.tile([H, BL], cdt, name="c0", tag="c0"),
             singles.tile([H, BL], cdt, name="c1", tag="c1")]
        h0_z = trans.tile([H, BL], f16, tag="h0", name="h0_init", bufs=3)
        nc.vector.memset(h0_z, 0.0)
        nc.vector.memset(h1, 0.0)
        for L in range(2):
            nc.vector.memset(c[L], 0.0)
        # layer-1 runs TWO steps behind layer-0: every ACT op in a period then
        # depends only on >= half-period-old results, so the h0 recurrence
        # tail (tanh -> h-mult -> PE -> first gate ACT) hides under L1's ops.
        h0_hist = [None, h0_z]

        ps_pg = ctx.enter_context(tc.tile_pool(name="ps_pg", bufs=3, space="PSUM"))
        ps_px = ctx.enter_context(tc.tile_pool(name="ps_px", bufs=2, space="PSUM"))

        def pg_tile(shape, name):
            return ps_pg.tile(shape, f32, tag="pg", name=name)

        # ---------------- main loop ----------------
        def lstm_step(L, inp, hprev, hout, hh_first, split=False,
                      first_gate_split=False):
            sig_i = trans.tile([H, BL], f16, tag="sig_i", name="sig_i")
            sig_f = trans.tile([H, BL], f16, tag="sig_f", name="sig_f")
            tg = trans.tile([H, BL], f16, tag="tg", name="tg")
            sig_o = trans.tile([H, BL], f16, tag="sig_o", name="sig_o")
            outs = [sig_i, sig_f, tg, sig_o]
            funcs = [AF.Sigmoid, AF.Sigmoid, AF.Tanh, AF.Sigmoid]
            wih = stat0 if L == 0 else wihT1
            for gc in range(4):
                pg = pg_tile([H, BL], "pg_gates")
                for hc in range(NH):
                    sl = slice(hc * 512, (hc + 1) * 512)
                    ops = [(wih[:, gc, :], inp), (whhT[L][:, gc, :], hprev)]
                    if hh_first:
                        ops.reverse()
                    nc.tensor.matmul(pg[:, sl], ops[0][0], ops[0][1][:, sl],
                                     start=True, stop=False)
                    nc.tensor.matmul(pg[:, sl], ops[1][0], ops[1][1][:, sl],
                                     start=False, stop=True)
                if gc == 0 and first_gate_split:
                    for hc in range(NH):
                        sl = slice(hc * 512, (hc + 1) * 512)
                        nc.scalar.activation(out=outs[gc][:, sl],
                                             in_=pg[:, sl], func=funcs[gc],
                                             bias=beff[L][:, gc:gc + 1],
                                             scale=1.0)
                else:
                    nc.scalar.activation(out=outs[gc], in_=pg, func=funcs[gc],
                                         bias=beff[L][:, gc:gc + 1], scale=1.0)
            u = trans.tile([H, BL], f16, tag="u", name="u")
            v_ = trans.tile([H, BL], cdt, tag="v_", name="v_")
            tc_ = trans.tile([H, BL], f16, tag="tc_", name="tc_")
            if not split:
                nc.vector.tensor_tensor(out=v_, in0=sig_f, in1=c[L], op=ALU.mult)
                nc.vector.tensor_tensor(out=u, in0=sig_i, in1=tg, op=ALU.mult)
                nc.vector.tensor_add(out=c[L], in0=u, in1=v_)
                nc.scalar.activation(out=tc_, in_=c[L], func=AF.Tanh, scale=1.0)
                nc.vector.tensor_tensor(out=hout, in0=sig_o, in1=tc_, op=ALU.mult)
            else:
                # half-column tail: lets tanh/h pipeline against the DVE chain
                # v-products first: they only need sig_f (2nd ACT op)
                for hc in range(NH):
                    sl = slice(hc * 512, (hc + 1) * 512)
                    nc.vector.tensor_tensor(out=v_[:, sl], in0=sig_f[:, sl],
                                            in1=c[L][:, sl], op=ALU.mult)
                for hc in range(NH):
                    sl = slice(hc * 512, (hc + 1) * 512)
                    nc.vector.tensor_tensor(out=u[:, sl], in0=sig_i[:, sl],
                                            in1=tg[:, sl], op=ALU.mult)
                    nc.vector.tensor_add(out=c[L][:, sl], in0=u[:, sl],
                                         in1=v_[:, sl])
                for hc in range(NH):
                    sl = slice(hc * 512, (hc + 1) * 512)
                    nc.scalar.activation(out=tc_[:, sl], in_=c[L][:, sl],
                                         func=AF.Tanh, scale=1.0)
                for hc in range(NH):
                    sl = slice(hc * 512, (hc + 1) * 512)
                    nc.vector.tensor_tensor(out=hout[:, sl], in0=sig_o[:, sl],
                                            in1=tc_[:, sl], op=ALU.mult)

        for t in range(T_steps):
            # scale x_t by rstd in batch-major layout (Pool), lane 7 = rstd
            xs = xsp.tile([128, QB, F + 1], f16, tag="xs", name="xs")
            nc.gpsimd.tensor_tensor(
                out=xs[:, :, 0:F], in0=xqh[:, :, t, :],
                in1=rT[:, :, t:t + 1].to_broadcast([128, QB, F]), op=ALU.mult)
            nc.gpsimd.tensor_tensor(out=xs[:, :, F], in0=rT[:, :, t],
                                    in1=ones_q, op=ALU.mult)
            # PE transpose to [8, BL] fp16 moving operand xt = [x r ; r]
            xt = trans.tile([F + 1, BL], f16, tag="xt", name="xt")
            for half in range(2):
                px = ps_px.tile([F + 1, 512], f16, tag="pxt", name="pxt")
                for qi in range(4):
                    q = half * 4 + qi
                    nc.tensor.transpose(px[:, qi * 128:(qi + 1) * 128],
                                        xs[:, q, :], ident16)
                nc.vector.tensor_copy(
                    out=xt[:, half * 512:(half + 1) * 512], in_=px)
            if t > 1:
                lstm_step(1, h0_hist[0], h1, h1, hh_first=True)
            h0_new = trans.tile([H, BL], f16, tag="h0", name="h0_new", bufs=3)
            lstm_step(0, xt, h0_hist[1], h0_new, hh_first=False, split=False)
            h0_hist = [h0_hist[1], h0_new]
            if dbg and t == 0:
                xtc = trans.tile([F + 1, BL], f32, tag="v_", name="xtc_dbg")
                nc.vector.tensor_copy(out=xtc, in_=xt)
                nc.sync.dma_start(out=dbg_xt[:, :], in_=xtc)
                h0c = trans.tile([H, BL], f32, tag="u", name="h0c_dbg")
                nc.vector.tensor_copy(out=h0c, in_=h0_new)
                nc.sync.dma_start(out=dbg_h0[:, :], in_=h0c)
                c0c = trans.tile([H, BL], f32, tag="tc_", name="c0c_dbg")
                nc.vector.tensor_copy(out=c0c, in_=c[0])
                nc.sync.dma_start(out=dbg_c0[:, :], in_=c0c)
        lstm_step(1, h0_hist[0], h1, h1, hh_first=True, split=True,
                  first_gate_split=True)
        lstm_step(1, h0_hist[1], h1, h1, hh_first=True, split=True,
                  first_gate_split=True)

        # ---------------- head ----------------
        sqh = trans.tile([H, BL], f16, tag="sig_f", name="sqh")
        nc.vector.tensor_tensor(out=sqh, in0=h1, in1=h1, op=ALU.mult)
        ps_s1 = pg_tile([1, BL], "ps_s1")
        ps_s2 = pg_tile([1, BL], "ps_s2")
        for hc in range(NH):
            sl = slice(hc * 512, (hc + 1) * 512)
            nc.tensor.matmul(ps_s1[:, sl], ones_col16, h1[:, sl],
                             start=True, stop=True, skip_group_check=True)
            nc.tensor.matmul(ps_s2[:, sl], ones_col16, sqh[:, sl],
                             start=True, stop=True, skip_group_check=True)
        # head LN stats: scale ops on ACT (f16 out), multiplies on DVE at 2x
        nm16 = singles.tile([1, BL], f16, tag="nm16", name="nm16")
        nc.scalar.activation(out=nm16, in_=ps_s1, func=AF.Copy,
                             scale=-1.0 / H)
        v16 = singles.tile([1, BL], f16, tag="v16", name="v16")
        nc.scalar.activation(out=v16, in_=ps_s2, func=AF.Copy, scale=1.0 / H)
        musq_h = singles.tile([1, BL], f16, tag="musq", name="musq_h")
        nc.vector.tensor_tensor(out=musq_h, in0=nm16, in1=nm16, op=ALU.mult)
        with nc.allow_low_precision(reason="head LN var in f16; |var|~O(1)"):
            nc.vector.tensor_sub(out=v16, in0=v16, in1=musq_h)
        rh16 = singles.tile([1, BL], f16, tag="rh16", name="rh16")
        nc.scalar.activation(out=rh16, in_=v16, func=AF.Sqrt,
                             bias=eps_col[0:1], scale=1.0)
        with nc.allow_low_precision(reason="head LN rstd in f16"):
            nc.vector.reciprocal(out=rh16, in_=rh16)
        pnm = pg_tile([H, BL], "pnm")
        prh = ps_px.tile([H, 512], f32, tag="pxt", name="prh0")
        prh2 = ps_px.tile([H, 512], f32, tag="pxt", name="prh1")
        prhs = [prh, prh2]
        for hc in range(NH):
            sl = slice(hc * 512, (hc + 1) * 512)
            nc.tensor.matmul(pnm[:, sl], ones_row128_16, nm16[:, sl],
                             start=True, stop=True, skip_group_check=True)
            nc.tensor.matmul(prhs[hc], ones_row128_16, rh16[:, sl],
                             start=True, stop=True, skip_group_check=True)
        t1 = trans.tile([H, BL], f32, tag="tg", name="t1")
        t2 = trans.tile([H, BL], f32, tag="sig_o", name="t2")
        last = trans.tile([H, BL], f16, tag="u", name="last")
        for hc in range(NH):
            sl = slice(hc * 512, (hc + 1) * 512)
            nc.vector.tensor_tensor(out=t1[:, sl], in0=h1[:, sl],
                                    in1=pnm[:, sl], op=ALU.add)
            nc.vector.tensor_tensor(out=t2[:, sl], in0=t1[:, sl], in1=prhs[hc],
                                    op=ALU.mult)
            nc.vector.tensor_scalar(out=last[:, sl], in0=t2[:, sl],
                                    scalar1=g_ln_c, scalar2=be_ln_c,
                                    op0=ALU.mult, op1=ALU.add)
        pd1 = pg_tile([D1, BL], "pd1")
        for hc in range(NH):
            sl = slice(hc * 512, (hc + 1) * 512)
            nc.tensor.matmul(pd1[:, sl], wd1T, last[:, sl], start=True, stop=True,
                             skip_group_check=True)
        d1 = trans.tile([D1, BL], f16, tag="v_", name="d1")
        for hc in range(NH):
            sl = slice(hc * 512, (hc + 1) * 512)
            nc.scalar.activation(out=d1[:, sl], in_=pd1[:, sl],
                                 func=AF.Relu, bias=b_d1_c, scale=1.0)
        pd2 = pg_tile([D2, BL], "pd2")
        for hc in range(NH):
            sl = slice(hc * 512, (hc + 1) * 512)
            nc.tensor.matmul(pd2[:, sl], wd2T, d1[:, sl], start=True, stop=True,
                             skip_group_check=True)
        d2 = trans.tile([D2, BL], f16, tag="tc_", name="d2")
        for hc in range(NH):
            sl = slice(hc * 512, (hc + 1) * 512)
            nc.scalar.activation(out=d2[:, sl], in_=pd2[:, sl],
                                 func=AF.Relu, bias=b_d2_c, scale=1.0)
        pd3 = pg_tile([OUT, BL], "pd3")
        for hc in range(NH):
            sl = slice(hc * 512, (hc + 1) * 512)
            nc.tensor.matmul(pd3[:, sl], wd3T, d2[:, sl], start=True, stop=True,
                             skip_group_check=True)
        o3 = trans.tile([OUT, BL], f32, tag="sig_f", name="o3")
        for hc in range(NH):
            sl = slice(hc * 512, (hc + 1) * 512)
            nc.scalar.activation(out=o3[:, sl], in_=pd3[:, sl],
                                 func=AF.Identity, bias=b_d3_c, scale=1.0)
        outT = singles.tile([128, QB, OUT], f32)
        for q in range(QB):
            pot = ps_px.tile([128, OUT], f32, tag="pxt", name="pot")
            nc.tensor.transpose(pot, o3[:, q * 128:(q + 1) * 128],
                                ident[:OUT, :OUT])
            nc.vector.tensor_copy(out=outT[:, q, :], in_=pot)
        nc.sync.dma_start(
            out=out_d[:, :].rearrange("(q p) c -> p q c", p=128),
            in_=outT)
    return nc


_CACHE = {}


def _get_runner(T_steps=T):
    if "runner" in _CACHE:
        return _CACHE["runner"]
    import jax
    from jax.sharding import Mesh, PartitionSpec
    from jax.experimental.shard_map import shard_map
    import concourse.bacc as bacc
    import concourse.mybir as mybir
    from concourse.bass2jax import install_neuronx_cc_hook, _bass_exec_p, \
        partition_id_tensor

    nc = bacc.Bacc()
    _build(nc, T_steps=T_steps)
    nc.compile()
    install_neuronx_cc_hook()

    partition_name = nc.partition_id_tensor.name if nc.partition_id_tensor else None
    in_names, out_names, out_avals, zero_outs = [], [], [], []
    for alloc in nc.m.functions[0].allocations:
        if not isinstance(alloc, mybir.MemoryLocationSet):
            continue
        name = alloc.memorylocations[0].name
        if alloc.kind == "ExternalInput":
            if name != partition_name:
                in_names.append(name)
        elif alloc.kind == "ExternalOutput":
            out_names.append(name)
            shape = tuple(alloc.tensor_shape)
            dtype = mybir.dt.np(alloc.dtype)
            out_avals.append(jax.core.ShapedArray(shape, dtype))
            zero_outs.append(np.zeros(shape, dtype))
    n_params = len(in_names)
    all_in_names = in_names + out_names + ([partition_name] if partition_name else [])

    def _body(*args):
        operands = list(args)
        if partition_name is not None:
            operands.append(partition_id_tensor())
        outs = _bass_exec_p.bind(
            *operands,
            out_avals=tuple(out_avals),
            in_names=tuple(all_in_names),
            out_names=tuple(out_names),
            lowering_input_output_aliases=(),
            sim_require_finite=False,
            sim_require_nnan=False,
            nc=nc,
        )
        return tuple(outs)

    devices = jax.devices()[:NCORES]
    mesh = Mesh(np.asarray(devices), ("core",))
    in_specs = (PartitionSpec("core"),) * (n_params + len(out_names))
    out_specs = (PartitionSpec("core"),) * len(out_names)
    sharded = jax.jit(
        shard_map(_body, mesh=mesh, in_specs=in_specs, out_specs=out_specs,
                  check_rep=False),
        keep_unused=True)
    _CACHE["runner"] = (sharded, in_names, out_names, zero_outs)
    return _CACHE["runner"]


def kernel(**inputs) -> np.ndarray:
    sharded, in_names, out_names, zero_outs = _get_runner()
    inp = {k: np.ascontiguousarray(np.asarray(v), dtype=np.float32)
           for k, v in inputs.items()}

    def core_val(name, ci):
        if name == "x":
            return inp["x"][ci * BL:(ci + 1) * BL]
        return inp[name]

    concat_in = [
        np.concatenate([core_val(n, ci) for ci in range(NCORES)], axis=0)
        for n in in_names
    ]
    concat_zeros = [
        np.zeros((NCORES * z.shape[0], *z.shape[1:]), z.dtype) for z in zero_outs
    ]
    import jax
    out_arrs = sharded(*concat_in, *concat_zeros)
    jax.block_until_ready(out_arrs)
    oi = out_names.index("out")
    full = np.asarray(out_arrs[oi]).reshape(B, OUT)
    return full.astype(np.float32)
